# revision 1
# baseline (speedup 1.0000x reference)
"""Trainium2 Bass kernel for nn_CryoformerDecoderLayer.

Sharding: 8 cores = 4 batches x 2 halves of the 512 residues.
Each core computes its 256 (residue, batch) rows end-to-end; the only
cross-core exchange is a pairwise AllGather of x0 (512KB) so each pair
can build full self-attention K/V for its batch. Host gather = concat.
"""

import os
import numpy as np
import ml_dtypes

import concourse.bass as bass
import concourse.mybir as mybir
import concourse.bacc as bacc
import concourse.tile as tile
from concourse.bass_utils import run_bass_kernel_spmd

F32 = mybir.dt.float32
F32R = mybir.dt.float32r
BF16 = mybir.dt.bfloat16
AF = mybir.ActivationFunctionType
ALU = mybir.AluOpType
AX = mybir.AxisListType

P = 128
D, H, FF, MSA, PAIR = 512, 8, 2048, 256, 128
NRES, B, NDEN = 512, 4, 4096
LLOC = 256
NC = 8
DH = D // H  # 64

# brows row indices
BR_MS, BR_PS, BR_SABV, BR_CABV, BR_SABO, BR_CABO, BR_B2 = range(7)
BR_LN = 7  # 7..18: g_ms, be_ms, g_ps, be_ps, g0, be0, g1, be1, g2, be2, g3, be3

LAST_EXEC_NS = None
_NC = None


def _r(ap):
    return ap.bitcast(F32R)


def _emit(nc, tc, drams):
    mm = nc.tensor.matmul

    from contextlib import ExitStack
    es = ExitStack()
    es.enter_context(nc.allow_low_precision(
        reason="float32r is 32-bit; tag only enables fast PE mode"))
    psp = es.enter_context(tc.tile_pool(name="psp", bufs=1, space="PSUM"))
    avp = es.enter_context(tc.tile_pool(name="avp", bufs=1, space="PSUM"))
    dram = es.enter_context(tc.tile_pool(name="dram", bufs=1, space="DRAM"))
    g = es.enter_context(tc.tile_pool(name="g", bufs=1))  # global sbuf pool

    def ps_tile(name):
        return psp.tile([P, 512], F32, name=name, tag="ps", bufs=3)

    def din(name):
        return drams[name].ap()

    # ---------------- persistents ----------------
    ones1 = g.tile([1, P], F32R, name="ones1")
    nc.sync.dma_start(ones1[:], din("onesr")[:, :])
    onescol = g.tile([P, 1], F32R, name="onescol")
    nc.sync.dma_start(onescol[:], din("onesc")[:, :])
    identity = g.tile([P, P], F32, name="identity")
    nc.sync.dma_start(identity[:], din("ident")[:, :])
    identityb = g.tile([P, P], BF16, name="identityb")
    nc.sync.dma_start(identityb[:], din("identb")[:, :])
    def brow(idx):
        t = g.tile([1, 512], F32R, name=f"brow{idx}", tag="brow", bufs=4)
        nc.sync.dma_start(t[:], din("brows")[idx:idx + 1, :])
        return t
    qb_sa = g.tile([P, 12], F32, name="qb_sa")
    nc.sync.dma_start(qb_sa[:], din("qb_sa")[:, :])
    qb_ca = g.tile([P, 12], F32, name="qb_ca")
    nc.sync.dma_start(qb_ca[:], din("qb_ca")[:, :])
    b1T = g.tile([P, 16], F32, name="b1T")
    nc.sync.dma_start(b1T[:], din("b1T")[:, :])

    def rep(idx):
        t = g.tile([P, 512], F32, name=f"rep{idx}", tag="rep", bufs=4)
        pt = ps_tile("prep")
        mm(pt[:, :], _r(ones1[:]), _r(brow(idx)[:]), start=True, stop=True)
        nc.scalar.copy(t[:], pt[:, :])
        return t

    def row_bias_mm(pt, idx):
        # add brows[idx] (a [512] row) onto every partition row of psum pt
        mm(pt[:, :], _r(ones1[:]), _r(brow(idx)[:]), start=False, stop=True)

    def ln(dst, src, g_ap, be_ap, pool):
        st6 = pool.tile([P, 6], F32, name="ln6", tag="ln6", bufs=3)
        nc.vector.bn_stats(st6[:], src)
        agg = pool.tile([P, 2], F32, name="ln2", tag="ln2", bufs=3)
        nc.vector.bn_aggr(agg[:], st6[:])
        nm = pool.tile([P, 1], F32, name="lnm", tag="lnm", bufs=3)
        nc.vector.tensor_scalar_mul(nm[:], agg[:, 0:1], -1.0)
        vr = pool.tile([P, 1], F32, name="lnv", tag="lnv", bufs=3)
        nc.vector.tensor_scalar_add(vr[:], agg[:, 1:2], 1e-5)
        rc = pool.tile([P, 1], F32, name="lnr", tag="lnr", bufs=3)
        nc.vector.reciprocal(rc[:], vr[:])
        rs = pool.tile([P, 1], F32, name="lns", tag="lns", bufs=3)
        nc.scalar.sqrt(rs[:], rc[:])
        xn = pool.tile([P, 512], F32, name="lnx", tag="lnx", bufs=3)
        nc.vector.tensor_scalar(xn[:], src, nm[:], rs[:], op0=ALU.add, op1=ALU.mult)
        nc.vector.tensor_mul(dst, xn[:], g_ap[:])
        nc.vector.tensor_add(dst, dst, be_ap[:])

    # residual-chain tiles (live across phases)
    x0 = g.tile([P, 2, 512], F32, name="x0")
    x1 = g.tile([P, 2, 512], F32, name="x1")
    x2 = g.tile([P, 2, 512], F32, name="x2")
    sa_acc = g.tile([P, 2, 512], F32, name="sa_acc")
    ca_acc = g.tile([P, 2, 512], F32, name="ca_acc")
    aa = g.tile([P, 2, 512], F32, name="aa")
    nc.sync.dma_start(aa[:], din("aa").rearrange("(lt p) d -> p lt d", p=P))
    x0T = g.tile([P, 4, 256], F32R, name="x0T")
    out_sb = g.tile([P, 2, 512], F32, name="out_sb")

    # ================= phase 0: pre-part =================
    with tc.tile_pool(name="p0", bufs=1) as p0:
        sgl = p0.tile([P, 2, 512], F32, name="sgl")
        nc.sync.dma_start(sgl[:], din("sgl").rearrange("(lt p) d -> p lt d", p=P))
        msa0T = p0.tile([P, 2, 256], F32R, name="msa0T")
        nc.sync.dma_start(msa0T[:], din("msa0T").rearrange("(kc p) l -> p kc l", p=P))
        WmsT = p0.tile([P, 2, 512], F32R, name="WmsT")
        nc.sync.dma_start(WmsT[:], din("WmsT").rearrange("(kc p) d -> p kc d", p=P))
        WpsT = p0.tile([P, 512], F32R, name="WpsT")
        nc.sync.dma_start(WpsT[:], din("WpsT")[:, :])

        xms = p0.tile([P, 2, 512], F32, name="xms")
        xps = p0.tile([P, 2, 512], F32, name="xps")

        g_ms = rep(BR_LN + 0)
        be_ms = rep(BR_LN + 1)
        for lt in range(2):
            pt = ps_tile("pms")
            for kc in range(2):
                mm(pt[:, :], _r(msa0T[:, kc, lt * P:(lt + 1) * P]),
                   _r(WmsT[:, kc, :]), start=(kc == 0), stop=False)
            row_bias_mm(pt, BR_MS)
            tmp = p0.tile([P, 512], F32, name="pre0", tag="pre", bufs=3)
            nc.vector.tensor_add(tmp[:], pt[:, :], sgl[:, lt, :])
            ln(xms[:, lt, :], tmp[:], g_ms, be_ms, p0)

        # pair mean: binary-tree adds (bf16 2x DVE mode) instead of 1x reduce
        pmeanT = p0.tile([P, 256], F32R, name="pmeanT")
        for i in range(32):
            pchunk = p0.tile([P, 8, 512], BF16, name="pchunk", tag="pchunk", bufs=3)
            nc.sync.dma_start(pchunk[:], din("parT")[:, i * 8:(i + 1) * 8, :])
            t = p0.tile([P, 8, 256], BF16, name="ptree", tag="ptree", bufs=2)
            nc.vector.tensor_add(t[:], pchunk[:, :, 0:256], pchunk[:, :, 256:512])
            w = 128
            while w >= 2:
                nc.vector.tensor_add(t[:, :, 0:w], t[:, :, 0:w], t[:, :, w:2 * w])
                w //= 2
            nc.vector.tensor_add(pmeanT[:, i * 8:(i + 1) * 8],
                                 t[:, :, 0:1], t[:, :, 1:2])

        g_ps = rep(BR_LN + 2)
        be_ps = rep(BR_LN + 3)
        for lt in range(2):
            pt = ps_tile("pps")
            mm(pt[:, :], _r(pmeanT[:, lt * P:(lt + 1) * P]), _r(WpsT[:]),
               start=True, stop=False)
            row_bias_mm(pt, BR_PS)
            tmp = p0.tile([P, 512], F32, name="pre1", tag="pre", bufs=3)
            nc.vector.tensor_add(tmp[:], pt[:, :], sgl[:, lt, :])
            ln(xps[:, lt, :], tmp[:], g_ps, be_ps, p0)

        g0 = rep(BR_LN + 4)
        be0 = rep(BR_LN + 5)
        for lt in range(2):
            tmp = p0.tile([P, 512], F32, name="pre2", tag="pre", bufs=3)
            nc.vector.tensor_add(tmp[:], xms[:, lt, :], xps[:, lt, :])
            ln(x0[:, lt, :], tmp[:], g0, be0, p0)

        # transpose x0 -> x0T
        for lt in range(2):
            for dc in range(4):
                tp = ps_tile("tp0")
                nc.tensor.transpose(tp[:P, :P], x0[:, lt, dc * P:(dc + 1) * P],
                                    identity[:])
                nc.scalar.copy(x0T[:, dc, lt * P:(lt + 1) * P], tp[:P, :P])

    # ================= allgather x0T within pairs =================
    snd = dram.tile([512, 256], F32R, name="snd")
    rcv = dram.tile([2, 512, 256], F32R, name="rcv")
    nc.sync.dma_start(snd.rearrange("(dc p) l -> p dc l", p=P), x0T[:])
    nc.gpsimd.collective_compute(
        "AllGather", ALU.bypass,
        replica_groups=[[0, 1], [2, 3], [4, 5], [6, 7]],
        ins=[snd.opt()], outs=[rcv.opt()],
    )

    # ================= phase 1: self-attention =================
    with tc.tile_pool(name="p1", bufs=1) as p1:
        x0fT = p1.tile([P, 4, 512], F32R, name="x0fT")
        for r in range(2):
            nc.sync.dma_start(
                x0fT[:, :, r * 256:(r + 1) * 256],
                rcv[r, :, :].rearrange("(dc p) l -> p dc l", p=P))
        aaT = p1.tile([P, 4, 512], F32R, name="aaT")
        nc.sync.dma_start(aaT[:], din("aaT").rearrange("(dc p) s -> p dc s", p=P))
        qkfT = p1.tile([P, 4, 512], F32R, name="qkfT")
        nc.vector.tensor_add(qkfT[:], x0fT[:], aaT[:])
        aaTl = p1.tile([P, 4, 256], F32R, name="aaTl")
        nc.sync.dma_start(aaTl[:], din("aaTl").rearrange("(dc p) l -> p dc l", p=P))
        qkTl = p1.tile([P, 4, 256], F32R, name="qkTl")
        nc.vector.tensor_add(qkTl[:], x0T[:], aaTl[:])

        saWqT = p1.tile([P, 4, 512], F32R, name="saWqT")
        nc.sync.dma_start(saWqT[:], din("saWqT").rearrange("(kc p) m -> p kc m", p=P))
        saWkT = p1.tile([P, 4, 512], F32R, name="saWkT")
        nc.sync.dma_start(saWkT[:], din("saWkT").rearrange("(kc p) m -> p kc m", p=P))
        saWvT = p1.tile([P, 4, 512], F32R, name="saWvT")
        nc.sync.dma_start(saWvT[:], din("saWvT").rearrange("(kc p) m -> p kc m", p=P))
        saWoT = p1.tile([64, 8, 512], F32R, name="saWoT")
        nc.sync.dma_start(saWoT[:], din("saWoT").rearrange("(h p) m -> p h m", p=64))

        QTsa = p1.tile([P, 4, 256], F32R, name="QTsa")
        for j in range(4):
            pt = ps_tile("pq")
            for kc in range(4):
                mm(pt[:, :256], _r(saWqT[:, kc, j * P:(j + 1) * P]),
                   _r(qkTl[:, kc, :]), start=(kc == 0), stop=(kc == 3))
            nc.scalar.add(QTsa[:, j, :], pt[:, :256], qb_sa[:, j:j + 1])
        KTsa = p1.tile([P, 4, 512], F32R, name="KTsa")
        for j in range(4):
            pt = ps_tile("pk")
            for kc in range(4):
                mm(pt[:, :], _r(saWkT[:, kc, j * P:(j + 1) * P]),
                   _r(qkfT[:, kc, :]), start=(kc == 0), stop=(kc == 3))
            nc.scalar.add(KTsa[:, j, :], pt[:, :], qb_sa[:, 4 + j:5 + j])
        Vsa = p1.tile([P, 4, 512], F32R, name="Vsa")
        for ms in range(4):
            pt = ps_tile("pv")
            for kc in range(4):
                mm(pt[:, :], _r(x0fT[:, kc, ms * P:(ms + 1) * P]),
                   _r(saWvT[:, kc, :]), start=(kc == 0), stop=False)
            row_bias_mm(pt, BR_SABV)
            nc.scalar.copy(Vsa[:, ms, :], pt[:, :])

        cs_sa = psp.tile([P, 2, 8], F32, name="cs_sa", tag="cs", bufs=1)
        av_sa = [avp.tile([64, 512], F32, name=f"avs{j}", tag="av", bufs=4)
                 for j in range(4)]
        for h in range(8):
            po, pc = (h % 2) * 64, h // 2
            for sc in range(4):
                pt = psp.tile([P, 512], F32, name="pst", tag="ps", bufs=3)[:, :256]
                mm(pt[:, :], _r(KTsa[po:po + 64, pc, sc * P:(sc + 1) * P]),
                   _r(QTsa[po:po + 64, pc, :]), start=True, stop=True)
                ex = p1.tile([P, 256], F32R, name="exs", tag="ex", bufs=4)
                nc.scalar.activation(ex[:], pt[:, :], AF.Exp)
                for lt in range(2):
                    mm(cs_sa[:, lt, h:h + 1],
                       ex[:, lt * P:(lt + 1) * P].bitcast(F32),
                       onescol.bitcast(F32), start=(sc == 0), stop=(sc == 3),
                       skip_group_check=True)
                mm(av_sa[h // 2][:, (h % 2) * 256:(h % 2 + 1) * 256],
                   _r(Vsa[:, sc, h * 64:(h + 1) * 64]), _r(ex[:]),
                   start=(sc == 0), stop=(sc == 3), skip_group_check=True)

        recip_sa = p1.tile([P, 2, 8], F32, name="recip_sa")
        nc.vector.reciprocal(recip_sa[:], cs_sa[:])
        for h in range(8):
            U = p1.tile([64, 256], F32R, name="Usa", tag="U", bufs=3)
            nc.scalar.copy(U[:], av_sa[h // 2][:, (h % 2) * 256:(h % 2 + 1) * 256])
            for lt in range(2):
                pt = ps_tile("pproj")
                mm(pt[:, :], _r(U[:, lt * P:(lt + 1) * P]),
                   _r(saWoT[:, h, :]), start=True, stop=True)
                if h == 0:
                    nc.vector.tensor_scalar(sa_acc[:, lt, :], pt[:, :],
                                            recip_sa[:, lt, h:h + 1], None,
                                            op0=ALU.mult)
                else:
                    nc.vector.scalar_tensor_tensor(
                        sa_acc[:, lt, :], pt[:, :], recip_sa[:, lt, h:h + 1],
                        sa_acc[:, lt, :], op0=ALU.mult, op1=ALU.add)

        g1 = rep(BR_LN + 6)
        be1 = rep(BR_LN + 7)
        bo_sa = rep(BR_SABO)
        for lt in range(2):
            tmp = p1.tile([P, 512], F32, name="pre3", tag="pre", bufs=3)
            nc.vector.tensor_add(tmp[:], x0[:, lt, :], sa_acc[:, lt, :])
            nc.vector.tensor_add(tmp[:], tmp[:], bo_sa[:])
            ln(x1[:, lt, :], tmp[:], g1, be1, p1)

    # ================= phase 2: cross-attention =================
    with tc.tile_pool(name="p2", bufs=1) as p2:
        caWqT = p2.tile([P, 4, 512], F32R, name="caWqT", tag="wproj", bufs=3)
        nc.sync.dma_start(caWqT[:], din("caWqT").rearrange("(kc p) m -> p kc m", p=P))
        caWkT = p2.tile([P, 4, 512], F32R, name="caWkT", tag="wproj", bufs=3)
        nc.sync.dma_start(caWkT[:], din("caWkT").rearrange("(kc p) m -> p kc m", p=P))
        caWvT = p2.tile([P, 4, 512], F32R, name="caWvT", tag="wproj", bufs=3)
        nc.sync.dma_start(caWvT[:], din("caWvT").rearrange("(kc p) m -> p kc m", p=P))
        caWoT = p2.tile([64, 8, 512], F32R, name="caWoT")
        nc.sync.dma_start(caWoT[:], din("caWoT").rearrange("(h p) m -> p h m", p=64))

        # queryT = (x1 + aa)^T
        qpre = p2.tile([P, 2, 512], F32, name="qpre")
        nc.vector.tensor_add(qpre[:], x1[:], aa[:])
        qT = p2.tile([P, 4, 256], F32R, name="qT")
        for lt in range(2):
            for dc in range(4):
                tp = ps_tile("tp1")
                nc.tensor.transpose(tp[:P, :P], qpre[:, lt, dc * P:(dc + 1) * P],
                                    identity[:])
                nc.scalar.copy(qT[:, dc, lt * P:(lt + 1) * P], tp[:P, :P])
        QTca = p2.tile([P, 4, 256], F32R, name="QTca")
        for j in range(4):
            pt = ps_tile("pq2")
            for kc in range(4):
                mm(pt[:, :256], _r(caWqT[:, kc, j * P:(j + 1) * P]),
                   _r(qT[:, kc, :]), start=(kc == 0), stop=(kc == 3))
            nc.scalar.add(QTca[:, j, :], pt[:, :256], qb_ca[:, j:j + 1])

        cs_ca = psp.tile([P, 2, 8], F32, name="cs_ca", tag="cs", bufs=1)
        av_ca = [avp.tile([64, 512], F32, name=f"avc{j}", tag="av", bufs=4)
                 for j in range(4)]

        NSC = 16  # density chunks of 256 rows
        for sc in range(NSC):
            s0 = sc * 256
            dT = p2.tile([P, 4, 256], F32R, name="dT", tag="dT", bufs=2)
            nc.sync.dma_start(
                dT[:], din("denT").rearrange("(dc p) s -> p dc s", p=P)
                [:, :, s0:s0 + 256])
            kmT = p2.tile([P, 4, 256], F32R, name="kmT", tag="kmT", bufs=2)
            nc.sync.dma_start(
                kmT[:], din("dposT").rearrange("(dc p) s -> p dc s", p=P)
                [:, :, s0:s0 + 256])
            nc.vector.tensor_add(kmT[:], kmT[:], dT[:])
            ktc = p2.tile([P, 4, 256], F32R, name="ktc", tag="ktc", bufs=2)
            for j in range(4):
                pt = psp.tile([P, 512], F32, name="pk2", tag="ps", bufs=3)[:, :256]
                for kc in range(4):
                    mm(pt[:, :], _r(caWkT[:, kc, j * P:(j + 1) * P]),
                       _r(kmT[:, kc, :]), start=(kc == 0), stop=(kc == 3))
                nc.scalar.add(ktc[:, j, :], pt[:, :], qb_ca[:, 4 + j:5 + j])
            vc = p2.tile([P, 2, 512], F32R, name="vc", tag="vc", bufs=2)
            for ms in range(2):
                pt = ps_tile("pv2")
                for kc in range(4):
                    mm(pt[:, :], _r(dT[:, kc, ms * P:(ms + 1) * P]),
                       _r(caWvT[:, kc, :]), start=(kc == 0), stop=False)
                row_bias_mm(pt, BR_CABV)
                nc.scalar.copy(vc[:, ms, :], pt[:, :])
            wei = p2.tile([P, 8, 2, 256], BF16, name="wei", tag="wei", bufs=2)
            for msd in range(2):
                nc.sync.dma_start(
                    wei[:, :, msd, :],
                    din("weiT")[:, s0 + msd * P:s0 + (msd + 1) * P, :]
                    .rearrange("h p l -> p h l"))
            first = (sc == 0)
            last = (sc == NSC - 1)
            for h in range(8):
                po, pc = (h % 2) * 64, h // 2
                for ms in range(2):
                    pt = psp.tile([P, 512], F32, name="pst2", tag="ps", bufs=3)[:, :256]
                    mm(pt[:, :], _r(ktc[po:po + 64, pc, ms * P:(ms + 1) * P]),
                       _r(QTca[po:po + 64, pc, :]), start=True, stop=False)
                    mm(pt[:, :], identityb[:], wei[:, h, ms, :],
                       start=False, stop=True)
                    ex = p2.tile([P, 256], F32R, name="exc", tag="ex", bufs=4)
                    nc.scalar.activation(ex[:], pt[:, :], AF.Exp)
                    for lt in range(2):
                        mm(cs_ca[:, lt, h:h + 1],
                           ex[:, lt * P:(lt + 1) * P].bitcast(F32),
                           onescol.bitcast(F32), start=(first and ms == 0),
                           stop=(last and ms == 1), skip_group_check=True)
                    mm(av_ca[h // 2][:, (h % 2) * 256:(h % 2 + 1) * 256],
                       _r(vc[:, ms, h * 64:(h + 1) * 64]), _r(ex[:]),
                       start=(first and ms == 0), stop=(last and ms == 1),
                       skip_group_check=True)

        recip_ca = p2.tile([P, 2, 8], F32, name="recip_ca")
        nc.vector.reciprocal(recip_ca[:], cs_ca[:])
        for h in range(8):
            U = p2.tile([64, 256], F32R, name="Uca", tag="U", bufs=3)
            nc.scalar.copy(U[:], av_ca[h // 2][:, (h % 2) * 256:(h % 2 + 1) * 256])
            for lt in range(2):
                pt = ps_tile("pproj2")
                mm(pt[:, :], _r(U[:, lt * P:(lt + 1) * P]),
                   _r(caWoT[:, h, :]), start=True, stop=True)
                if h == 0:
                    nc.vector.tensor_scalar(ca_acc[:, lt, :], pt[:, :],
                                            recip_ca[:, lt, h:h + 1], None,
                                            op0=ALU.mult)
                else:
                    nc.vector.scalar_tensor_tensor(
                        ca_acc[:, lt, :], pt[:, :], recip_ca[:, lt, h:h + 1],
                        ca_acc[:, lt, :], op0=ALU.mult, op1=ALU.add)

        g2 = rep(BR_LN + 8)
        be2 = rep(BR_LN + 9)
        bo_ca = rep(BR_CABO)
        for lt in range(2):
            tmp = p2.tile([P, 512], F32, name="pre4", tag="pre", bufs=3)
            nc.vector.tensor_add(tmp[:], x1[:, lt, :], ca_acc[:, lt, :])
            nc.vector.tensor_add(tmp[:], tmp[:], bo_ca[:])
            ln(x2[:, lt, :], tmp[:], g2, be2, p2)

    # ================= phase 3: FFN =================
    with tc.tile_pool(name="p3", bufs=1) as p3:
        W1T = p3.tile([P, 4, 2048], F32R, name="W1T", tag="wff", bufs=2)
        nc.sync.dma_start(W1T[:], din("W1T").rearrange("(kc p) m -> p kc m", p=P))
        W2T = p3.tile([P, 16, 512], F32R, name="W2T", tag="wff", bufs=2)
        nc.sync.dma_start(W2T[:], din("W2T").rearrange("(kc p) m -> p kc m", p=P))

        x2T = p3.tile([P, 4, 256], F32R, name="x2T")
        for lt in range(2):
            for dc in range(4):
                tp = ps_tile("tp2")
                nc.tensor.transpose(tp[:P, :P], x2[:, lt, dc * P:(dc + 1) * P],
                                    identity[:])
                nc.scalar.copy(x2T[:, dc, lt * P:(lt + 1) * P], tp[:P, :P])

        fT = p3.tile([P, 16, 256], F32R, name="fT")
        for j in range(16):
            pt = ps_tile("pf")
            for kc in range(4):
                mm(pt[:, :256], _r(W1T[:, kc, j * P:(j + 1) * P]),
                   _r(x2T[:, kc, :]), start=(kc == 0), stop=(kc == 3))
            nc.scalar.activation(fT[:, j, :], pt[:, :256], AF.Relu,
                                 bias=b1T[:, j:j + 1])

        g3 = rep(BR_LN + 10)
        be3 = rep(BR_LN + 11)
        for lt in range(2):
            pt = ps_tile("pff")
            for j in range(16):
                mm(pt[:, :], _r(fT[:, j, lt * P:(lt + 1) * P]),
                   _r(W2T[:, j, :]), start=(j == 0), stop=False)
            row_bias_mm(pt, BR_B2)
            tmp = p3.tile([P, 512], F32, name="pre5", tag="pre", bufs=3)
            nc.vector.tensor_add(tmp[:], pt[:, :], x2[:, lt, :])
            ln(out_sb[:, lt, :], tmp[:], g3, be3, p3)

    nc.sync.dma_start(din("out").rearrange("(lt p) d -> p lt d", p=P), out_sb[:])

    es.close()


def _build():
    nc = bacc.Bacc("TRN2", target_bir_lowering=False, debug=False, num_devices=NC)
    specs = [
        ("msa0T", [MSA, LLOC], F32R),
        ("sgl", [LLOC, D], F32),
        ("parT", [PAIR, LLOC, NRES], BF16),
        ("aa", [LLOC, D], F32),
        ("aaT", [D, NRES], F32R),
        ("aaTl", [D, LLOC], F32R),
        ("denT", [D, NDEN], F32R),
        ("dposT", [D, NDEN], F32R),
        ("weiT", [H, NDEN, LLOC], BF16),
        ("WmsT", [MSA, D], F32R),
        ("WpsT", [PAIR, D], F32R),
        ("saWqT", [D, D], F32R),
        ("saWkT", [D, D], F32R),
        ("saWvT", [D, D], F32R),
        ("saWoT", [D, D], F32R),
        ("caWqT", [D, D], F32R),
        ("caWkT", [D, D], F32R),
        ("caWvT", [D, D], F32R),
        ("caWoT", [D, D], F32R),
        ("W1T", [D, FF], F32R),
        ("W2T", [FF, D], F32R),
        ("qb_sa", [P, 12], F32),
        ("qb_ca", [P, 12], F32),
        ("b1T", [P, 16], F32),
        ("brows", [19, D], F32R),
        ("onesr", [1, P], F32R),
        ("onesc", [P, 1], F32R),
        ("ident", [P, P], F32),
        ("identb", [P, P], BF16),
    ]
    drams = {}
    for name, shape, dt in specs:
        drams[name] = nc.dram_tensor(name, shape, dt, kind="ExternalInput")
    drams["out"] = nc.dram_tensor("out", [LLOC, D], F32, kind="ExternalOutput")

    with tile.TileContext(nc) as tc:
        _emit(nc, tc, drams)
    nc.compile()
    return nc


def _prep_core_inputs(inputs, b, half):
    L0 = half * LLOC
    f32 = np.float32
    bf16 = ml_dtypes.bfloat16

    def C(a, dt=f32):
        return np.ascontiguousarray(a, dtype=dt)

    tgt_msa = inputs["tgt_msa"]
    tgt_sgl = inputs["tgt_sgl"]
    tgt_par = inputs["tgt_par"]
    aa_embed = inputs["aa_embed"]
    density_repr = inputs["density_repr"]
    den_pos = inputs["den_pos"]
    den_wei = inputs["den_wei"]

    m = {}
    m["msa0T"] = C(tgt_msa[0, b, L0:L0 + LLOC, :].T)
    m["sgl"] = C(tgt_sgl[L0:L0 + LLOC, b])
    m["parT"] = C(tgt_par[L0:L0 + LLOC, b].transpose(2, 0, 1), bf16)
    m["aa"] = C(aa_embed[L0:L0 + LLOC, b])
    m["aaT"] = C(aa_embed[:, b].T)
    m["aaTl"] = C(aa_embed[L0:L0 + LLOC, b].T)
    m["denT"] = C(density_repr[:, b].T)
    m["dposT"] = C(den_pos[:, b].T)
    m["weiT"] = C((8.0 * den_wei[b * H:(b + 1) * H, L0:L0 + LLOC, :])
                  .transpose(0, 2, 1), bf16)
    return m


def _prep_shared_inputs(inputs):
    f32 = np.float32

    def C(a):
        return np.ascontiguousarray(a, dtype=f32)

    m = {}
    m["WmsT"] = C(inputs["W_ms"].T)
    m["WpsT"] = C(inputs["W_ps"].T / NRES)
    sa_W = np.asarray(inputs["sa_Wqkv"], f32)
    m["saWqT"] = C(sa_W[:D].T / 8.0)
    m["saWkT"] = C(sa_W[D:2 * D].T)
    m["saWvT"] = C(sa_W[2 * D:].T)
    m["saWoT"] = C(inputs["sa_Wo"].T)
    ca_W = np.asarray(inputs["ca_Wqkv"], f32)
    m["caWqT"] = C(ca_W[:D].T / 8.0)
    m["caWkT"] = C(ca_W[D:2 * D].T)
    m["caWvT"] = C(ca_W[2 * D:].T)
    m["caWoT"] = C(inputs["ca_Wo"].T)
    m["W1T"] = C(inputs["W1"].T)
    m["W2T"] = C(inputs["W2"].T)

    sa_b = np.asarray(inputs["sa_bqkv"], f32).copy()
    sa_b[:D] /= 8.0
    m["qb_sa"] = C(sa_b.reshape(12, P).T)
    ca_b = np.asarray(inputs["ca_bqkv"], f32).copy()
    ca_b[:D] /= 8.0
    m["qb_ca"] = C(ca_b.reshape(12, P).T)
    m["b1T"] = C(np.asarray(inputs["b1"], f32).reshape(16, P).T)

    brows = np.stack([
        inputs["b_ms"], inputs["b_ps"],
        sa_b[2 * D:], np.asarray(inputs["ca_bqkv"], f32)[2 * D:],
        inputs["sa_bo"], inputs["ca_bo"], inputs["b2"],
        inputs["g_ms"], inputs["be_ms"], inputs["g_ps"], inputs["be_ps"],
        inputs["g0"], inputs["be0"], inputs["g1"], inputs["be1"],
        inputs["g2"], inputs["be2"], inputs["g3"], inputs["be3"],
    ]).astype(f32)
    m["brows"] = C(brows)
    m["onesr"] = np.ones((1, P), f32)
    m["onesc"] = np.ones((P, 1), f32)
    m["ident"] = np.eye(P, dtype=f32)
    m["identb"] = np.eye(P, dtype=ml_dtypes.bfloat16)
    return m


def kernel(**inputs):
    global _NC, LAST_EXEC_NS
    inputs = {k: np.asarray(v) for k, v in inputs.items()}
    if _NC is None:
        _NC = _build()
    nc = _NC

    shared = _prep_shared_inputs(inputs)
    in_maps = []
    for c in range(NC):
        m = _prep_core_inputs(inputs, c // 2, c % 2)
        m.update(shared)
        in_maps.append(m)

    trace = bool(os.environ.get("BASS_TRACE"))
    res = run_bass_kernel_spmd(nc, in_maps, core_ids=list(range(NC)), trace=trace)
    LAST_EXEC_NS = res.exec_time_ns

    out = np.empty((NRES, B, D), np.float32)
    for c in range(NC):
        b, half = c // 2, c % 2
        out[half * LLOC:(half + 1) * LLOC, b] = res.results[c]["out"]
    return out



# revision 7
# speedup vs baseline: 1.8322x; 1.8322x over previous
"""Trainium2 Bass kernel for nn_CryoformerDecoderLayer.

Sharding: 8 cores = 4 batches x 2 halves of the 512 residues.
Each core computes its 256 (residue, batch) rows end-to-end; the only
cross-core exchange is a pairwise AllGather of x0 (256KB bf16) so each
pair can build full self-attention K/V for its batch.

Perf notes vs v1:
- all matmuls bf16 (FWL fast weight loads, half DMA traffic)
- parT streamed as fp8 (pair-mean branch is a ~1% perturbation)
- softmax denominators via a ones-column appended to V (row 64 of the
  attention-numerator PSUM accumulates sum(exp)) instead of 576
  moving-dim-1 matmuls
- gaussian bias added on DVE instead of an identity matmul
- K-bias dropped (softmax shift invariance, exact); V-bias folded into
  the out-proj bias on host (exact)
- all weights prefetched at kernel start
"""

import os
import numpy as np
import ml_dtypes

import concourse.bass as bass
import concourse.mybir as mybir
import concourse.bacc as bacc
import concourse.tile as tile
from concourse.bass_utils import run_bass_kernel_spmd

F32 = mybir.dt.float32
F32R = mybir.dt.float32r
BF16 = mybir.dt.bfloat16
F8 = mybir.dt.float8e4
AF = mybir.ActivationFunctionType
ALU = mybir.AluOpType

P = 128
D, H, FF, MSA, PAIR = 512, 8, 2048, 256, 128
NRES, B, NDEN = 512, 4, 4096
LLOC = 256
NC = 8
DH = D // H  # 64

# brows row indices
BR_MS, BR_PS, BR_SABO, BR_CABO, BR_B2 = range(5)
BR_LN = 5  # 5..16: g_ms, be_ms, g_ps, be_ps, g0, be0, g1, be1, g2, be2, g3, be3

LAST_EXEC_NS = None
_NC = None


def _r(ap):
    return ap.bitcast(F32R)


def _emit(nc, tc, drams):
    mm = nc.tensor.matmul

    from contextlib import ExitStack
    es = ExitStack()
    es.enter_context(nc.allow_low_precision(
        reason="bf16 matmuls are fine at rel-tol 2e-2"))
    psp = es.enter_context(tc.tile_pool(name="psp", bufs=1, space="PSUM"))
    avp = es.enter_context(tc.tile_pool(name="avp", bufs=1, space="PSUM"))
    dram = es.enter_context(tc.tile_pool(name="dram", bufs=1, space="DRAM"))
    g = es.enter_context(tc.tile_pool(name="g", bufs=1))  # global sbuf pool

    def ps_tile(name):
        return psp.tile([P, 512], F32, name=name, tag="ps", bufs=3)

    def din(name):
        return drams[name].ap()

    # ---------------- persistents / prefetched weights ----------------
    onesr = g.tile([1, P], F32R, name="onesr")
    nc.sync.dma_start(onesr[:], din("onesr")[:, :])
    ident = g.tile([P, P], F32, name="ident")
    nc.sync.dma_start(ident[:], din("ident")[:, :])

    def brow(idx):
        t = g.tile([1, 512], F32R, name=f"brow{idx}", tag="brow", bufs=4)
        nc.sync.dma_start(t[:], din("brows")[idx:idx + 1, :])
        return t

    def rep(idx):
        t = g.tile([P, 512], F32, name=f"rep{idx}", tag="rep", bufs=4)
        pt = ps_tile("prep")
        mm(pt[:, :], _r(onesr[:]), _r(brow(idx)[:]), start=True, stop=True)
        nc.scalar.copy(t[:], pt[:, :])
        return t

    def row_bias_mm(pt, idx):
        # add brows[idx] (a [512] row) onto every partition row of psum pt
        mm(pt[:, :], _r(onesr[:]), _r(brow(idx)[:]), start=False, stop=True)

    qb_sa = g.tile([P, 4], F32, name="qb_sa")
    nc.sync.dma_start(qb_sa[:], din("qb_sa")[:, :])
    qb_ca = g.tile([P, 4], F32, name="qb_ca")
    nc.sync.dma_start(qb_ca[:], din("qb_ca")[:, :])
    b1T = g.tile([P, 16], F32, name="b1T")
    nc.sync.dma_start(b1T[:], din("b1T")[:, :])

    WmsT = g.tile([P, 2, 512], BF16, name="WmsT")
    nc.sync.dma_start(WmsT[:], din("WmsT").rearrange("(kc p) d -> p kc d", p=P))
    WpsT = g.tile([P, 512], BF16, name="WpsT")
    nc.sync.dma_start(WpsT[:], din("WpsT")[:, :])

    def wload(name, shape, pat, p=P):
        t = g.tile(shape, BF16, name=name)
        nc.sync.dma_start(t[:], din(name).rearrange(pat, p=p))
        return t

    saWqT = wload("saWqT", [P, 4, 512], "(kc p) m -> p kc m")
    saWkT = wload("saWkT", [P, 4, 512], "(kc p) m -> p kc m")
    saWvT = wload("saWvT", [P, 4, 512], "(kc p) m -> p kc m")
    saWoT = wload("saWoT", [64, 8, 512], "(h p) m -> p h m", p=64)
    caWqT = wload("caWqT", [P, 4, 512], "(kc p) m -> p kc m")
    caWkT = wload("caWkT", [P, 4, 512], "(kc p) m -> p kc m")
    caWvT = wload("caWvT", [P, 4, 512], "(kc p) m -> p kc m")
    caWoT = wload("caWoT", [64, 8, 512], "(h p) m -> p h m", p=64)
    W1T = wload("W1T", [P, 4, 2048], "(kc p) m -> p kc m")
    W2T = wload("W2T", [P, 16, 512], "(kc p) m -> p kc m")

    aa = g.tile([P, 2, 512], F32, name="aa")
    nc.sync.dma_start(aa[:], din("aa").rearrange("(lt p) d -> p lt d", p=P))
    aaT = g.tile([P, 4, 512], BF16, name="aaT")
    nc.sync.dma_start(aaT[:], din("aaT").rearrange("(dc p) s -> p dc s", p=P))
    aaTl = g.tile([P, 4, 256], BF16, name="aaTl")
    nc.sync.dma_start(aaTl[:], din("aaTl").rearrange("(dc p) l -> p dc l", p=P))

    def ln(dst, src, g_ap, be_ap, pool):
        st6 = pool.tile([P, 6], F32, name="ln6", tag="ln6", bufs=3)
        nc.vector.bn_stats(st6[:], src)
        agg = pool.tile([P, 2], F32, name="ln2", tag="ln2", bufs=3)
        nc.vector.bn_aggr(agg[:], st6[:])
        nm = pool.tile([P, 1], F32, name="lnm", tag="lnm", bufs=3)
        nc.vector.tensor_scalar_mul(nm[:], agg[:, 0:1], -1.0)
        vr = pool.tile([P, 1], F32, name="lnv", tag="lnv", bufs=3)
        nc.vector.tensor_scalar_add(vr[:], agg[:, 1:2], 1e-5)
        rc = pool.tile([P, 1], F32, name="lnr", tag="lnr", bufs=3)
        nc.vector.reciprocal(rc[:], vr[:])
        rs = pool.tile([P, 1], F32, name="lns", tag="lns", bufs=3)
        nc.scalar.sqrt(rs[:], rc[:])
        xn = pool.tile([P, 512], F32, name="lnx", tag="lnx", bufs=3)
        nc.vector.tensor_scalar(xn[:], src, nm[:], rs[:], op0=ALU.add, op1=ALU.mult)
        nc.vector.tensor_mul(dst, xn[:], g_ap[:])
        nc.vector.tensor_add(dst, dst, be_ap[:])

    # residual-chain tiles (live across phases)
    x0 = g.tile([P, 2, 512], F32, name="x0")
    x1 = g.tile([P, 2, 512], F32, name="x1")
    x2 = g.tile([P, 2, 512], F32, name="x2")
    x0T = g.tile([P, 4, 256], BF16, name="x0T")
    out_sb = g.tile([P, 2, 512], F32, name="out_sb")

    # ================= phase 0: pre-part =================
    with tc.tile_pool(name="p0", bufs=1) as p0:
        sgl = p0.tile([P, 2, 512], F32, name="sgl")
        nc.sync.dma_start(sgl[:], din("sgl").rearrange("(lt p) d -> p lt d", p=P))
        msa0T = p0.tile([P, 2, 256], BF16, name="msa0T")
        nc.sync.dma_start(msa0T[:], din("msa0T").rearrange("(kc p) l -> p kc l", p=P))

        xms = p0.tile([P, 2, 512], F32, name="xms")
        xps = p0.tile([P, 2, 512], F32, name="xps")

        g_ms = rep(BR_LN + 0)
        be_ms = rep(BR_LN + 1)
        for lt in range(2):
            pt = ps_tile("pms")
            for kc in range(2):
                mm(pt[:, :], msa0T[:, kc, lt * P:(lt + 1) * P],
                   WmsT[:, kc, :], start=(kc == 0), stop=False)
            row_bias_mm(pt, BR_MS)
            tmp = p0.tile([P, 512], F32, name="pre0", tag="pre", bufs=3)
            nc.vector.tensor_add(tmp[:], pt[:, :], sgl[:, lt, :])
            ln(xms[:, lt, :], tmp[:], g_ms, be_ms, p0)

        # pair mean: fp8 stream, binary-tree adds in bf16
        pmeanT = p0.tile([P, 256], BF16, name="pmeanT")
        for i in range(32):
            pchunk = p0.tile([P, 8, 512], F8, name="pchunk", tag="pchunk", bufs=3)
            nc.sync.dma_start(pchunk[:], din("parT")[:, i * 8:(i + 1) * 8, :])
            t = p0.tile([P, 8, 256], BF16, name="ptree", tag="ptree", bufs=2)
            nc.vector.tensor_add(t[:], pchunk[:, :, 0:256], pchunk[:, :, 256:512])
            w = 128
            while w >= 2:
                nc.vector.tensor_add(t[:, :, 0:w], t[:, :, 0:w], t[:, :, w:2 * w])
                w //= 2
            nc.vector.tensor_add(pmeanT[:, i * 8:(i + 1) * 8],
                                 t[:, :, 0:1], t[:, :, 1:2])

        g_ps = rep(BR_LN + 2)
        be_ps = rep(BR_LN + 3)
        for lt in range(2):
            pt = ps_tile("pps")
            mm(pt[:, :], pmeanT[:, lt * P:(lt + 1) * P], WpsT[:],
               start=True, stop=False)
            row_bias_mm(pt, BR_PS)
            tmp = p0.tile([P, 512], F32, name="pre1", tag="pre", bufs=3)
            nc.vector.tensor_add(tmp[:], pt[:, :], sgl[:, lt, :])
            ln(xps[:, lt, :], tmp[:], g_ps, be_ps, p0)

        g0 = rep(BR_LN + 4)
        be0 = rep(BR_LN + 5)
        for lt in range(2):
            tmp = p0.tile([P, 512], F32, name="pre2", tag="pre", bufs=3)
            nc.vector.tensor_add(tmp[:], xms[:, lt, :], xps[:, lt, :])
            ln(x0[:, lt, :], tmp[:], g0, be0, p0)

        # transpose x0 -> x0T (bf16)
        for lt in range(2):
            for dc in range(4):
                tp = ps_tile("tp0")
                nc.tensor.transpose(tp[:P, :P], x0[:, lt, dc * P:(dc + 1) * P],
                                    ident[:])
                nc.scalar.copy(x0T[:, dc, lt * P:(lt + 1) * P], tp[:P, :P])

    # ================= allgather x0T within pairs =================
    snd = dram.tile([512, 256], BF16, name="snd")
    rcv = dram.tile([2, 512, 256], BF16, name="rcv")
    nc.sync.dma_start(snd.rearrange("(dc p) l -> p dc l", p=P), x0T[:])
    nc.gpsimd.collective_compute(
        "AllGather", ALU.bypass,
        replica_groups=[[0, 1], [2, 3], [4, 5], [6, 7]],
        ins=[snd.opt()], outs=[rcv.opt()],
    )

    # ================= phase 1: self-attention =================
    with tc.tile_pool(name="p1", bufs=1) as p1:
        x0fT = p1.tile([P, 4, 512], BF16, name="x0fT")
        for r in range(2):
            nc.sync.dma_start(
                x0fT[:, :, r * 256:(r + 1) * 256],
                rcv[r, :, :].rearrange("(dc p) l -> p dc l", p=P))
        qkfT = p1.tile([P, 4, 512], BF16, name="qkfT")
        nc.vector.tensor_add(qkfT[:], x0fT[:], aaT[:])
        qkTl = p1.tile([P, 4, 256], BF16, name="qkTl")
        nc.vector.tensor_add(qkTl[:], x0T[:], aaTl[:])

        # Q stored per-head zero-padded to full 128 contraction rows so the
        # score matmuls never need base-partition-64 operands (bf16 matmuls
        # at base partition 64 hang the device).
        QTsa = p1.tile([P, 8, 256], BF16, name="QTsa")
        nc.vector.memset(QTsa[:], 0.0)
        for j in range(4):
            pt = ps_tile("pq")
            for kc in range(4):
                mm(pt[:, :256], saWqT[:, kc, j * P:(j + 1) * P],
                   qkTl[:, kc, :], start=(kc == 0), stop=(kc == 3))
            nc.scalar.add(QTsa[0:64, 2 * j, :], pt[0:64, :256],
                          qb_sa[0:64, j:j + 1])
            nc.scalar.add(QTsa[64:128, 2 * j + 1, :], pt[64:128, :256],
                          qb_sa[64:128, j:j + 1])
        KTsa = p1.tile([P, 4, 512], BF16, name="KTsa")
        for j in range(4):
            pt = ps_tile("pk")
            for kc in range(4):
                mm(pt[:, :], saWkT[:, kc, j * P:(j + 1) * P],
                   qkfT[:, kc, :], start=(kc == 0), stop=(kc == 3))
            nc.vector.tensor_scalar_add(KTsa[:, j, :], pt[:, :], 0.0)
        # V with ones column appended per head (row 64 of AV = sum(exp))
        Vsa = p1.tile([P, 4, 8, 65], BF16, name="Vsa")
        nc.gpsimd.memset(Vsa[:, :, :, 64:65], 1.0)
        for ms in range(4):
            pt = ps_tile("pv")
            for kc in range(4):
                mm(pt[:, :], x0fT[:, kc, ms * P:(ms + 1) * P],
                   saWvT[:, kc, :], start=(kc == 0), stop=(kc == 3))
            nc.vector.tensor_scalar_add(
                Vsa[:, ms, :, 0:64],
                pt[:, :].rearrange("p (h d) -> p h d", h=8), 0.0)

        av_sa = [avp.tile([65, 512], F32, name=f"avs{j}", tag="av", bufs=4)
                 for j in range(4)]
        for sc in range(4):
            for hp in range(4):
                pt = ps_tile("pst")
                mm(pt[:, 0:256], KTsa[:, hp, sc * P:(sc + 1) * P],
                   QTsa[:, 2 * hp, :], start=True, stop=True,
                   skip_group_check=True)
                mm(pt[:, 256:512], KTsa[:, hp, sc * P:(sc + 1) * P],
                   QTsa[:, 2 * hp + 1, :], start=True, stop=True,
                   skip_group_check=True)
                ex = p1.tile([P, 2, 256], BF16, name="exs", tag="ex", bufs=4)
                nc.scalar.activation(
                    ex[:], pt[:, :].rearrange("p (h l) -> p h l", h=2), AF.Exp)
                for h2 in range(2):
                    h = 2 * hp + h2
                    mm(av_sa[hp][:, h2 * 256:(h2 + 1) * 256],
                       Vsa[:, sc, h, :], ex[:, h2, :],
                       start=(sc == 0), stop=(sc == 3), skip_group_check=True)

        # denominators: row 64 of each av bank -> [128L, 16] psum -> recip
        dns = p1.tile([65, 4, 512], F32, name="dns")
        for hp in range(4):
            nc.scalar.copy(dns[64:65, hp, :], av_sa[hp][64:65, :])
        rc_ps = ps_tile("rcps")
        for hp in range(4):
            for h2 in range(2):
                for lt in range(2):
                    col = lt * 8 + 2 * hp + h2
                    mm(rc_ps[:, col:col + 1],
                       dns[64:65, hp, h2 * 256 + lt * P:h2 * 256 + (lt + 1) * P],
                       ident[64:65, 64:65], start=True, stop=True,
                       skip_group_check=True)
        recip_sa = p1.tile([P, 2, 8], F32, name="recip_sa")
        nc.vector.reciprocal(
            recip_sa[:], rc_ps[:, 0:16].rearrange("p (lt h) -> p lt h", lt=2))

        sa_acc = p1.tile([P, 2, 512], F32, name="sa_acc")
        for hp in range(4):
            for h2 in range(2):
                h = 2 * hp + h2
                U = p1.tile([64, 256], BF16, name="Usa", tag="U", bufs=3)
                nc.scalar.copy(U[:], av_sa[hp][0:64, h2 * 256:(h2 + 1) * 256])
                for lt in range(2):
                    pt = ps_tile("pproj")
                    mm(pt[:, :], U[:, lt * P:(lt + 1) * P],
                       saWoT[:, h, :], start=True, stop=True)
                    if h == 0:
                        nc.vector.tensor_scalar(sa_acc[:, lt, :], pt[:, :],
                                                recip_sa[:, lt, h:h + 1], None,
                                                op0=ALU.mult)
                    else:
                        nc.vector.scalar_tensor_tensor(
                            sa_acc[:, lt, :], pt[:, :], recip_sa[:, lt, h:h + 1],
                            sa_acc[:, lt, :], op0=ALU.mult, op1=ALU.add)

        g1 = rep(BR_LN + 6)
        be1 = rep(BR_LN + 7)
        bo_sa = rep(BR_SABO)
        for lt in range(2):
            tmp = p1.tile([P, 512], F32, name="pre3", tag="pre", bufs=3)
            nc.vector.tensor_add(tmp[:], x0[:, lt, :], sa_acc[:, lt, :])
            nc.vector.tensor_add(tmp[:], tmp[:], bo_sa[:])
            ln(x1[:, lt, :], tmp[:], g1, be1, p1)

    # ================= phase 2: cross-attention =================
    with tc.tile_pool(name="p2", bufs=1) as p2:
        # queryT = (x1 + aa)^T
        qpre = p2.tile([P, 2, 512], F32, name="qpre")
        nc.vector.tensor_add(qpre[:], x1[:], aa[:])
        qT = p2.tile([P, 4, 256], BF16, name="qT")
        for lt in range(2):
            for dc in range(4):
                tp = ps_tile("tp1")
                nc.tensor.transpose(tp[:P, :P], qpre[:, lt, dc * P:(dc + 1) * P],
                                    ident[:])
                nc.scalar.copy(qT[:, dc, lt * P:(lt + 1) * P], tp[:P, :P])
        QTca = p2.tile([P, 8, 256], BF16, name="QTca")
        nc.vector.memset(QTca[:], 0.0)
        for j in range(4):
            pt = ps_tile("pq2")
            for kc in range(4):
                mm(pt[:, :256], caWqT[:, kc, j * P:(j + 1) * P],
                   qT[:, kc, :], start=(kc == 0), stop=(kc == 3))
            nc.scalar.add(QTca[0:64, 2 * j, :], pt[0:64, :256],
                          qb_ca[0:64, j:j + 1])
            nc.scalar.add(QTca[64:128, 2 * j + 1, :], pt[64:128, :256],
                          qb_ca[64:128, j:j + 1])

        av_ca = [avp.tile([65, 512], F32, name=f"avc{j}", tag="av", bufs=4)
                 for j in range(4)]

        NSC = 16  # density chunks of 256 rows
        kmTap = din("kmT").rearrange("(dc p) s -> p dc s", p=P)
        denTap = din("denT").rearrange("(dc p) s -> p dc s", p=P)
        for sc in range(NSC):
            s0 = sc * 256
            kmc = p2.tile([P, 4, 256], BF16, name="kmc", tag="kmc", bufs=3)
            nc.sync.dma_start(kmc[:], kmTap[:, :, s0:s0 + 256])
            dc_t = p2.tile([P, 4, 256], BF16, name="dct", tag="dct", bufs=3)
            nc.sync.dma_start(dc_t[:], denTap[:, :, s0:s0 + 256])
            wei = p2.tile([P, 2, 4, 2, 256], BF16, name="wei", tag="wei", bufs=2)
            nc.sync.dma_start(wei[:], din("weiT")[sc])

            ktc = p2.tile([P, 4, 256], BF16, name="ktc", tag="ktc", bufs=2)
            for j in range(4):
                pt = ps_tile("pk2")
                for kc in range(4):
                    mm(pt[:, :256], caWkT[:, kc, j * P:(j + 1) * P],
                       kmc[:, kc, :], start=(kc == 0), stop=(kc == 3))
                nc.vector.tensor_scalar_add(ktc[:, j, :], pt[:, :256], 0.0)
            vc = p2.tile([P, 2, 8, 65], BF16, name="vc", tag="vc", bufs=2)
            nc.gpsimd.memset(vc[:, :, :, 64:65], 1.0)
            for ms in range(2):
                pt = ps_tile("pv2")
                for kc in range(4):
                    mm(pt[:, :], dc_t[:, kc, ms * P:(ms + 1) * P],
                       caWvT[:, kc, :], start=(kc == 0), stop=(kc == 3))
                nc.vector.tensor_scalar_add(
                    vc[:, ms, :, 0:64],
                    pt[:, :].rearrange("p (h d) -> p h d", h=8), 0.0)

            first = (sc == 0)
            last = (sc == NSC - 1)
            for ms in range(2):
                for hp in range(4):
                    pt = ps_tile("pst2")
                    mm(pt[:, 0:256], ktc[:, hp, ms * P:(ms + 1) * P],
                       QTca[:, 2 * hp, :], start=True, stop=True,
                       skip_group_check=True)
                    mm(pt[:, 256:512], ktc[:, hp, ms * P:(ms + 1) * P],
                       QTca[:, 2 * hp + 1, :], start=True, stop=True,
                       skip_group_check=True)
                    sb = p2.tile([P, 2, 256], F32, name="sb", tag="sb", bufs=3)
                    nc.vector.tensor_add(
                        sb[:], pt[:, :].rearrange("p (h l) -> p h l", h=2),
                        wei[:, ms, hp])
                    ex = p2.tile([P, 2, 256], BF16, name="exc", tag="ex", bufs=4)
                    nc.scalar.activation(ex[:], sb[:], AF.Exp)
                    for h2 in range(2):
                        h = 2 * hp + h2
                        mm(av_ca[hp][:, h2 * 256:(h2 + 1) * 256],
                           vc[:, ms, h, :], ex[:, h2, :],
                           start=(first and ms == 0), stop=(last and ms == 1),
                           skip_group_check=True)

        dns2 = p2.tile([65, 4, 512], F32, name="dns2")
        for hp in range(4):
            nc.scalar.copy(dns2[64:65, hp, :], av_ca[hp][64:65, :])
        rc_ps2 = ps_tile("rcps2")
        for hp in range(4):
            for h2 in range(2):
                for lt in range(2):
                    col = lt * 8 + 2 * hp + h2
                    mm(rc_ps2[:, col:col + 1],
                       dns2[64:65, hp, h2 * 256 + lt * P:h2 * 256 + (lt + 1) * P],
                       ident[64:65, 64:65], start=True, stop=True,
                       skip_group_check=True)
        recip_ca = p2.tile([P, 2, 8], F32, name="recip_ca")
        nc.vector.reciprocal(
            recip_ca[:], rc_ps2[:, 0:16].rearrange("p (lt h) -> p lt h", lt=2))

        ca_acc = p2.tile([P, 2, 512], F32, name="ca_acc")
        for hp in range(4):
            for h2 in range(2):
                h = 2 * hp + h2
                U = p2.tile([64, 256], BF16, name="Uca", tag="U", bufs=3)
                nc.scalar.copy(U[:], av_ca[hp][0:64, h2 * 256:(h2 + 1) * 256])
                for lt in range(2):
                    pt = ps_tile("pproj2")
                    mm(pt[:, :], U[:, lt * P:(lt + 1) * P],
                       caWoT[:, h, :], start=True, stop=True)
                    if h == 0:
                        nc.vector.tensor_scalar(ca_acc[:, lt, :], pt[:, :],
                                                recip_ca[:, lt, h:h + 1], None,
                                                op0=ALU.mult)
                    else:
                        nc.vector.scalar_tensor_tensor(
                            ca_acc[:, lt, :], pt[:, :], recip_ca[:, lt, h:h + 1],
                            ca_acc[:, lt, :], op0=ALU.mult, op1=ALU.add)

        g2 = rep(BR_LN + 8)
        be2 = rep(BR_LN + 9)
        bo_ca = rep(BR_CABO)
        for lt in range(2):
            tmp = p2.tile([P, 512], F32, name="pre4", tag="pre", bufs=3)
            nc.vector.tensor_add(tmp[:], x1[:, lt, :], ca_acc[:, lt, :])
            nc.vector.tensor_add(tmp[:], tmp[:], bo_ca[:])
            ln(x2[:, lt, :], tmp[:], g2, be2, p2)

    # ================= phase 3: FFN =================
    with tc.tile_pool(name="p3", bufs=1) as p3:
        x2T = p3.tile([P, 4, 256], BF16, name="x2T")
        for lt in range(2):
            for dc in range(4):
                tp = ps_tile("tp2")
                nc.tensor.transpose(tp[:P, :P], x2[:, lt, dc * P:(dc + 1) * P],
                                    ident[:])
                nc.scalar.copy(x2T[:, dc, lt * P:(lt + 1) * P], tp[:P, :P])

        fT = p3.tile([P, 16, 256], BF16, name="fT")
        for j in range(16):
            pt = ps_tile("pf")
            for kc in range(4):
                mm(pt[:, :256], W1T[:, kc, j * P:(j + 1) * P],
                   x2T[:, kc, :], start=(kc == 0), stop=(kc == 3))
            nc.scalar.activation(fT[:, j, :], pt[:, :256], AF.Relu,
                                 bias=b1T[:, j:j + 1])

        g3 = rep(BR_LN + 10)
        be3 = rep(BR_LN + 11)
        for lt in range(2):
            pt = ps_tile("pff")
            for j in range(16):
                mm(pt[:, :], fT[:, j, lt * P:(lt + 1) * P],
                   W2T[:, j, :], start=(j == 0), stop=False)
            row_bias_mm(pt, BR_B2)
            tmp = p3.tile([P, 512], F32, name="pre5", tag="pre", bufs=3)
            nc.vector.tensor_add(tmp[:], pt[:, :], x2[:, lt, :])
            ln(out_sb[:, lt, :], tmp[:], g3, be3, p3)

    nc.sync.dma_start(din("out").rearrange("(lt p) d -> p lt d", p=P), out_sb[:])

    es.close()


def _build():
    nc = bacc.Bacc("TRN2", target_bir_lowering=False, debug=False, num_devices=NC)
    specs = [
        ("msa0T", [MSA, LLOC], BF16),
        ("sgl", [LLOC, D], F32),
        ("parT", [PAIR, LLOC, NRES], F8),
        ("aa", [LLOC, D], F32),
        ("aaT", [D, NRES], BF16),
        ("aaTl", [D, LLOC], BF16),
        ("denT", [D, NDEN], BF16),
        ("kmT", [D, NDEN], BF16),
        ("weiT", [16, P, 2, 4, 2, LLOC], BF16),
        ("WmsT", [MSA, D], BF16),
        ("WpsT", [PAIR, D], BF16),
        ("saWqT", [D, D], BF16),
        ("saWkT", [D, D], BF16),
        ("saWvT", [D, D], BF16),
        ("saWoT", [D, D], BF16),
        ("caWqT", [D, D], BF16),
        ("caWkT", [D, D], BF16),
        ("caWvT", [D, D], BF16),
        ("caWoT", [D, D], BF16),
        ("W1T", [D, FF], BF16),
        ("W2T", [FF, D], BF16),
        ("qb_sa", [P, 4], F32),
        ("qb_ca", [P, 4], F32),
        ("b1T", [P, 16], F32),
        ("brows", [17, D], F32R),
        ("onesr", [1, P], F32R),
        ("ident", [P, P], F32),
    ]
    drams = {}
    for name, shape, dt in specs:
        drams[name] = nc.dram_tensor(name, shape, dt, kind="ExternalInput")
    drams["out"] = nc.dram_tensor("out", [LLOC, D], F32, kind="ExternalOutput")

    with tile.TileContext(nc) as tc:
        _emit(nc, tc, drams)
    nc.compile()
    return nc


def _prep_core_inputs(inputs, b, half):
    L0 = half * LLOC
    f32 = np.float32
    bf16 = ml_dtypes.bfloat16
    f8 = ml_dtypes.float8_e4m3

    def C(a, dt=f32):
        return np.ascontiguousarray(a, dtype=dt)

    tgt_msa = inputs["tgt_msa"]
    tgt_sgl = inputs["tgt_sgl"]
    tgt_par = inputs["tgt_par"]
    aa_embed = inputs["aa_embed"]
    density_repr = inputs["density_repr"]
    den_pos = inputs["den_pos"]
    den_wei = inputs["den_wei"]

    m = {}
    m["msa0T"] = C(tgt_msa[0, b, L0:L0 + LLOC, :].T, bf16)
    m["sgl"] = C(tgt_sgl[L0:L0 + LLOC, b])
    m["parT"] = C(tgt_par[L0:L0 + LLOC, b].transpose(2, 0, 1), f8)
    m["aa"] = C(aa_embed[L0:L0 + LLOC, b])
    m["aaT"] = C(aa_embed[:, b].T, bf16)
    m["aaTl"] = C(aa_embed[L0:L0 + LLOC, b].T, bf16)
    m["denT"] = C(density_repr[:, b].T, bf16)
    m["kmT"] = C((density_repr[:, b] + den_pos[:, b]).T, bf16)
    # weiT[sc, p, ms, hp, h2, l] = 8*den_wei[b*H + 2*hp + h2, L0+l, sc*256+ms*128+p]
    w = (8.0 * den_wei[b * H:(b + 1) * H, L0:L0 + LLOC, :]).astype(f32)
    # w: [8, 256, 4096] -> [4, 2, 256, 16, 2, 128] -> [16, 128, 2, 4, 2, 256]
    w = w.reshape(4, 2, LLOC, 16, 2, P).transpose(3, 5, 4, 0, 1, 2)
    m["weiT"] = C(w, bf16)
    return m


def _prep_shared_inputs(inputs):
    f32 = np.float32
    bf16 = ml_dtypes.bfloat16

    def C(a, dt=bf16):
        return np.ascontiguousarray(a, dtype=dt)

    m = {}
    m["WmsT"] = C(inputs["W_ms"].T)
    m["WpsT"] = C(np.asarray(inputs["W_ps"], f32).T / NRES)
    sa_W = np.asarray(inputs["sa_Wqkv"], f32)
    m["saWqT"] = C(sa_W[:D].T / 8.0)
    m["saWkT"] = C(sa_W[D:2 * D].T)
    m["saWvT"] = C(sa_W[2 * D:].T)
    m["saWoT"] = C(np.asarray(inputs["sa_Wo"], f32).T)
    ca_W = np.asarray(inputs["ca_Wqkv"], f32)
    m["caWqT"] = C(ca_W[:D].T / 8.0)
    m["caWkT"] = C(ca_W[D:2 * D].T)
    m["caWvT"] = C(ca_W[2 * D:].T)
    m["caWoT"] = C(np.asarray(inputs["ca_Wo"], f32).T)
    m["W1T"] = C(np.asarray(inputs["W1"], f32).T)
    m["W2T"] = C(np.asarray(inputs["W2"], f32).T)

    sa_b = np.asarray(inputs["sa_bqkv"], f32)
    ca_b = np.asarray(inputs["ca_bqkv"], f32)
    m["qb_sa"] = C((sa_b[:D] / 8.0).reshape(4, P).T, f32)
    m["qb_ca"] = C((ca_b[:D] / 8.0).reshape(4, P).T, f32)
    m["b1T"] = C(np.asarray(inputs["b1"], f32).reshape(16, P).T, f32)

    # V-bias folded into out-proj bias: softmax weights sum to 1
    bo_sa = np.asarray(inputs["sa_bo"], f32) + sa_b[2 * D:] @ np.asarray(
        inputs["sa_Wo"], f32).T
    bo_ca = np.asarray(inputs["ca_bo"], f32) + ca_b[2 * D:] @ np.asarray(
        inputs["ca_Wo"], f32).T

    brows = np.stack([
        inputs["b_ms"], inputs["b_ps"], bo_sa, bo_ca, inputs["b2"],
        inputs["g_ms"], inputs["be_ms"], inputs["g_ps"], inputs["be_ps"],
        inputs["g0"], inputs["be0"], inputs["g1"], inputs["be1"],
        inputs["g2"], inputs["be2"], inputs["g3"], inputs["be3"],
    ]).astype(f32)
    m["brows"] = C(brows, f32)
    m["onesr"] = np.ones((1, P), f32)
    m["ident"] = np.eye(P, dtype=f32)
    return m


def kernel(**inputs):
    global _NC, LAST_EXEC_NS
    inputs = {k: np.asarray(v) for k, v in inputs.items()}
    if _NC is None:
        _NC = _build()
    nc = _NC

    shared = _prep_shared_inputs(inputs)
    in_maps = []
    for c in range(NC):
        m = _prep_core_inputs(inputs, c // 2, c % 2)
        m.update(shared)
        in_maps.append(m)

    trace = bool(os.environ.get("BASS_TRACE"))
    res = run_bass_kernel_spmd(nc, in_maps, core_ids=list(range(NC)), trace=trace)
    LAST_EXEC_NS = res.exec_time_ns

    out = np.empty((NRES, B, D), np.float32)
    for c in range(NC):
        b, half = c // 2, c % 2
        out[half * LLOC:(half + 1) * LLOC, b] = res.results[c]["out"]
    return out


# revision 18
# speedup vs baseline: 1.8643x; 1.0175x over previous
"""Trainium2 Bass kernel for nn_CryoformerDecoderLayer.

Sharding: 8 cores = 4 batches x 2 halves of the 512 residues.
Each core computes its 256 (residue, batch) rows end-to-end; the only
cross-core exchange is a pairwise AllGather of x0 (256KB bf16) so each
pair can build full self-attention K/V for its batch.

Perf notes vs v1:
- all matmuls bf16 (FWL fast weight loads, half DMA traffic)
- parT streamed as fp8 (pair-mean branch is a ~1% perturbation)
- softmax denominators via a ones-column appended to V (row 64 of the
  attention-numerator PSUM accumulates sum(exp)) instead of 576
  moving-dim-1 matmuls
- gaussian bias added on DVE instead of an identity matmul
- K-bias dropped (softmax shift invariance, exact); V-bias folded into
  the out-proj bias on host (exact)
- all weights prefetched at kernel start
"""

import os
import numpy as np
import ml_dtypes

import concourse.bass as bass
import concourse.mybir as mybir
import concourse.bacc as bacc
import concourse.tile as tile
from concourse.bass_utils import run_bass_kernel_spmd

F32 = mybir.dt.float32
F32R = mybir.dt.float32r
BF16 = mybir.dt.bfloat16
F8 = mybir.dt.float8e4
AF = mybir.ActivationFunctionType
ALU = mybir.AluOpType

P = 128
D, H, FF, MSA, PAIR = 512, 8, 2048, 256, 128
NRES, B, NDEN = 512, 4, 4096
LLOC = 256
NC = 8
DH = D // H  # 64

# brows row indices
BR_MS, BR_PS, BR_SABO, BR_CABO, BR_B2 = range(5)
BR_LN = 5  # 5..16: g_ms, be_ms, g_ps, be_ps, g0, be0, g1, be1, g2, be2, g3, be3

LAST_EXEC_NS = None
_NC = None


def _r(ap):
    return ap.bitcast(F32R)


def _emit(nc, tc, drams):
    mm = nc.tensor.matmul

    from contextlib import ExitStack
    es = ExitStack()
    es.enter_context(nc.allow_low_precision(
        reason="bf16 matmuls are fine at rel-tol 2e-2"))
    psp = es.enter_context(tc.tile_pool(name="psp", bufs=1, space="PSUM"))
    avp = es.enter_context(tc.tile_pool(name="avp", bufs=1, space="PSUM"))
    dram = es.enter_context(tc.tile_pool(name="dram", bufs=1, space="DRAM"))
    g = es.enter_context(tc.tile_pool(name="g", bufs=1))  # global sbuf pool

    def ps_tile(name):
        return psp.tile([P, 512], F32, name=name, tag="ps", bufs=4)

    def din(name):
        return drams[name].ap()

    # ---------------- persistents / prefetched weights ----------------
    onesr = g.tile([1, P], F32R, name="onesr")
    nc.sync.dma_start(onesr[:], din("onesr")[:, :])
    ident = g.tile([P, P], F32, name="ident")
    nc.sync.dma_start(ident[:], din("ident")[:, :])

    def brow(idx):
        t = g.tile([1, 512], F32R, name=f"brow{idx}", tag="brow", bufs=4)
        nc.sync.dma_start(t[:], din("brows")[idx:idx + 1, :])
        return t

    def rep(idx):
        t = g.tile([P, 512], F32, name=f"rep{idx}", tag="rep", bufs=3)
        pt = ps_tile("prep")
        mm(pt[:, :], _r(onesr[:]), _r(brow(idx)[:]), start=True, stop=True)
        nc.scalar.copy(t[:], pt[:, :])
        return t

    def row_bias_mm(pt, idx):
        # add brows[idx] (a [512] row) onto every partition row of psum pt
        mm(pt[:, :], _r(onesr[:]), _r(brow(idx)[:]), start=False, stop=True)

    qb_sa = g.tile([P, 4], F32, name="qb_sa")
    nc.sync.dma_start(qb_sa[:], din("qb_sa")[:, :])
    qb_ca = g.tile([P, 4], F32, name="qb_ca")
    nc.sync.dma_start(qb_ca[:], din("qb_ca")[:, :])
    b1T = g.tile([P, 16], F32, name="b1T")
    nc.sync.dma_start(b1T[:], din("b1T")[:, :])

    WmsT = g.tile([P, 2, 512], BF16, name="WmsT")
    nc.sync.dma_start(WmsT[:], din("WmsT").rearrange("(kc p) d -> p kc d", p=P))
    WpsT = g.tile([P, 512], BF16, name="WpsT")
    nc.sync.dma_start(WpsT[:], din("WpsT")[:, :])

    def wload(pool, name, shape, pat, p=P):
        t = pool.tile(shape, BF16, name=name)
        nc.sync.dma_start(t[:], din(name).rearrange(pat, p=p))
        return t

    caWqT = wload(g, "caWqT", [P, 4, 512], "(kc p) m -> p kc m")
    caWkT = wload(g, "caWkT", [P, 4, 512], "(kc p) m -> p kc m")
    caWvT = wload(g, "caWvT", [P, 4, 512], "(kc p) m -> p kc m")
    caWoT = wload(g, "caWoT", [64, 8, 512], "(h p) m -> p h m", p=64)

    aa = g.tile([P, 2, 512], F32, name="aa")
    nc.sync.dma_start(aa[:], din("aa").rearrange("(lt p) d -> p lt d", p=P))
    aaTl = g.tile([P, 4, 256], BF16, name="aaTl")
    nc.sync.dma_start(aaTl[:], din("aaTl").rearrange("(dc p) l -> p dc l", p=P))

    def ln(dst, src, g_ap, be_ap, pool):
        st6 = pool.tile([P, 6], F32, name="ln6", tag="ln6", bufs=3)
        nc.vector.bn_stats(st6[:], src)
        agg = pool.tile([P, 2], F32, name="ln2", tag="ln2", bufs=3)
        nc.vector.bn_aggr(agg[:], st6[:])
        nm = pool.tile([P, 1], F32, name="lnm", tag="lnm", bufs=3)
        nc.vector.tensor_scalar_mul(nm[:], agg[:, 0:1], -1.0)
        vr = pool.tile([P, 1], F32, name="lnv", tag="lnv", bufs=3)
        nc.vector.tensor_scalar_add(vr[:], agg[:, 1:2], 1e-5)
        rc = pool.tile([P, 1], F32, name="lnr", tag="lnr", bufs=3)
        nc.vector.reciprocal(rc[:], vr[:])
        rs = pool.tile([P, 1], F32, name="lns", tag="lns", bufs=3)
        nc.scalar.sqrt(rs[:], rc[:])
        xn = pool.tile([P, 512], F32, name="lnx", tag="lnx", bufs=3)
        nc.vector.tensor_scalar(xn[:], src, nm[:], rs[:], op0=ALU.add, op1=ALU.mult)
        nc.vector.tensor_mul(dst, xn[:], g_ap[:])
        nc.vector.tensor_add(dst, dst, be_ap[:])

    # residual-chain tiles: 2-buffer rotation (x2 reuses x0's buffer once
    # x0 is dead after the SA layernorm; out_sb reuses x1's)
    x0 = g.tile([P, 2, 512], F32, name="x0", tag="xres", bufs=2)
    x1 = g.tile([P, 2, 512], F32, name="x1", tag="xres", bufs=2)
    x0T = g.tile([P, 4, 256], BF16, name="x0T")
    dns = g.tile([65, 4, 512], F32, name="dns")

    x2 = None
    out_sb = None
    # CA K/V precomputed during phase 0 (PE is otherwise idle there)
    ktcF = g.tile([P, 4, NDEN], BF16, name="ktcF")
    vcF = g.tile([P, 32, 8, 65], BF16, name="vcF")
    nc.vector.memset(vcF[:, :, :, 64:65], 1.0)

    saw_cm = tc.tile_pool(name="saw", bufs=1)
    saw = saw_cm.__enter__()
    saWqT = wload(saw, "saWqT", [P, 4, 512], "(kc p) m -> p kc m")
    saWkT = wload(saw, "saWkT", [P, 4, 512], "(kc p) m -> p kc m")
    saWvT = wload(saw, "saWvT", [P, 4, 512], "(kc p) m -> p kc m")
    saWoT = wload(saw, "saWoT", [64, 8, 512], "(h p) m -> p h m", p=64)

    # ================= phase 0: pre-part =================
    with tc.tile_pool(name="p0", bufs=1) as p0:
        sgl = p0.tile([P, 2, 512], F32, name="sgl")
        nc.sync.dma_start(sgl[:], din("sgl").rearrange("(lt p) d -> p lt d", p=P))
        msa0T = p0.tile([P, 2, 256], BF16, name="msa0T")
        nc.sync.dma_start(msa0T[:], din("msa0T").rearrange("(kc p) l -> p kc l", p=P))

        xms = p0.tile([P, 2, 512], F32, name="xms")
        xps = p0.tile([P, 2, 512], F32, name="xps")

        g_ms = rep(BR_LN + 0)
        be_ms = rep(BR_LN + 1)
        for lt in range(2):
            pt = ps_tile("pms")
            for kc in range(2):
                mm(pt[:, :], msa0T[:, kc, lt * P:(lt + 1) * P],
                   WmsT[:, kc, :], start=(kc == 0), stop=False)
            row_bias_mm(pt, BR_MS)
            tmp = p0.tile([P, 512], F32, name="pre0", tag="pre", bufs=3)
            nc.vector.tensor_add(tmp[:], pt[:, :], sgl[:, lt, :])
            ln(xms[:, lt, :], tmp[:], g_ms, be_ms, p0)

        # CA K/V projections over all density chunks (fills PE during the
        # DVE/DMA-bound pair-mean stream)
        kmTap = din("kmT").rearrange("(dc p) s -> p dc s", p=P)
        denTap = din("denT").rearrange("(dc p) s -> p dc s", p=P)
        for kb in range(16):
            s0 = kb * 256
            kmc = p0.tile([P, 4, 256], BF16, name="kmc", tag="kmc", bufs=2)
            nc.sync.dma_start(kmc[:], kmTap[:, :, s0:s0 + 256])
            dc_t = p0.tile([P, 4, 256], BF16, name="dct", tag="dct", bufs=2)
            nc.sync.dma_start(dc_t[:], denTap[:, :, s0:s0 + 256])
            for j in range(4):
                pt = ps_tile("pk2")
                for kc in range(4):
                    mm(pt[:, :256], caWkT[:, kc, j * P:(j + 1) * P],
                       kmc[:, kc, :], start=(kc == 0), stop=(kc == 3))
                nc.scalar.copy(ktcF[:, j, s0:s0 + 256], pt[:, :256])
            for ms in range(2):
                pt = ps_tile("pv2")
                for kc in range(4):
                    mm(pt[:, :], dc_t[:, kc, ms * P:(ms + 1) * P],
                       caWvT[:, kc, :], start=(kc == 0), stop=(kc == 3))
                nc.scalar.copy(
                    vcF[:, 2 * kb + ms, :, 0:64],
                    pt[:, :].rearrange("p (h d) -> p h d", h=8))

        # pair mean: fp8 in HBM, cast to bf16 by the gpsimd DMA datapath,
        # binary-tree adds in bf16 on DVE
        pmeanT = p0.tile([P, 256], BF16, name="pmeanT")
        for i in range(64):
            pchunk = p0.tile([P, 4, 512], BF16, name="pchunk", tag="pchunk", bufs=2)
            nc.gpsimd.dma_start(pchunk[:], din("parT")[:, i * 4:(i + 1) * 4, :])
            t = p0.tile([P, 4, 256], BF16, name="ptree", tag="ptree", bufs=2)
            nc.vector.tensor_add(t[:], pchunk[:, :, 0:256], pchunk[:, :, 256:512])
            w = 128
            while w >= 2:
                nc.vector.tensor_add(t[:, :, 0:w], t[:, :, 0:w], t[:, :, w:2 * w])
                w //= 2
            nc.vector.tensor_add(pmeanT[:, i * 4:(i + 1) * 4],
                                 t[:, :, 0:1], t[:, :, 1:2])

        g_ps = rep(BR_LN + 2)
        be_ps = rep(BR_LN + 3)
        for lt in range(2):
            pt = ps_tile("pps")
            mm(pt[:, :], pmeanT[:, lt * P:(lt + 1) * P], WpsT[:],
               start=True, stop=False)
            row_bias_mm(pt, BR_PS)
            tmp = p0.tile([P, 512], F32, name="pre1", tag="pre", bufs=3)
            nc.vector.tensor_add(tmp[:], pt[:, :], sgl[:, lt, :])
            ln(xps[:, lt, :], tmp[:], g_ps, be_ps, p0)

        g0 = rep(BR_LN + 4)
        be0 = rep(BR_LN + 5)
        for lt in range(2):
            tmp = p0.tile([P, 512], F32, name="pre2", tag="pre", bufs=3)
            nc.vector.tensor_add(tmp[:], xms[:, lt, :], xps[:, lt, :])
            ln(x0[:, lt, :], tmp[:], g0, be0, p0)

        # transpose x0 -> x0T (bf16)
        for lt in range(2):
            for dc in range(4):
                tp = ps_tile("tp0")
                nc.tensor.transpose(tp[:P, :P], x0[:, lt, dc * P:(dc + 1) * P],
                                    ident[:])
                nc.scalar.copy(x0T[:, dc, lt * P:(lt + 1) * P], tp[:P, :P])

    # ================= allgather x0T within pairs =================
    snd = dram.tile([512, 256], BF16, name="snd")
    rcv = dram.tile([2, 512, 256], BF16, name="rcv")
    nc.sync.dma_start(snd.rearrange("(dc p) l -> p dc l", p=P), x0T[:])
    nc.gpsimd.collective_compute(
        "AllGather", ALU.bypass,
        replica_groups=[[0, 1], [2, 3], [4, 5], [6, 7]],
        ins=[snd.opt()], outs=[rcv.opt()],
    )

    # ================= phase 1: self-attention =================
    with tc.tile_pool(name="p1", bufs=1) as p1:
        # local Q first: doesn't need the allgather result
        qkTl = p1.tile([P, 4, 256], BF16, name="qkTl")
        nc.vector.tensor_add(qkTl[:], x0T[:], aaTl[:])
        # Q stored per-head zero-padded to full 128 contraction rows so the
        # score matmuls never need base-partition-64 operands (bf16 matmuls
        # at base partition 64 hang the device).
        QTsa = p1.tile([P, 8, 256], BF16, name="QTsa")
        nc.vector.memset(QTsa[:], 0.0)
        for j in range(4):
            pt = ps_tile("pq")
            for kc in range(4):
                mm(pt[:, :256], saWqT[:, kc, j * P:(j + 1) * P],
                   qkTl[:, kc, :], start=(kc == 0), stop=(kc == 3))
            nc.scalar.add(QTsa[0:64, 2 * j, :], pt[0:64, :256],
                          qb_sa[0:64, j:j + 1])
            nc.scalar.add(QTsa[64:128, 2 * j + 1, :], pt[64:128, :256],
                          qb_sa[64:128, j:j + 1])

        aaT = p1.tile([P, 4, 512], BF16, name="aaT")
        nc.sync.dma_start(aaT[:], din("aaT").rearrange("(dc p) s -> p dc s", p=P))
        x0fT = p1.tile([P, 4, 512], BF16, name="x0fT")
        for r in range(2):
            nc.sync.dma_start(
                x0fT[:, :, r * 256:(r + 1) * 256],
                rcv[r, :, :].rearrange("(dc p) l -> p dc l", p=P))
        qkfT = p1.tile([P, 4, 512], BF16, name="qkfT")
        nc.vector.tensor_add(qkfT[:], x0fT[:], aaT[:])
        KTsa = p1.tile([P, 4, 512], BF16, name="KTsa")
        for j in range(4):
            pt = ps_tile("pk")
            for kc in range(4):
                mm(pt[:, :], saWkT[:, kc, j * P:(j + 1) * P],
                   qkfT[:, kc, :], start=(kc == 0), stop=(kc == 3))
            nc.vector.tensor_scalar_add(KTsa[:, j, :], pt[:, :], 0.0)
        # V with ones column appended per head (row 64 of AV = sum(exp))
        Vsa = p1.tile([P, 4, 8, 65], BF16, name="Vsa")
        nc.gpsimd.memset(Vsa[:, :, :, 64:65], 1.0)
        for ms in range(4):
            pt = ps_tile("pv")
            for kc in range(4):
                mm(pt[:, :], x0fT[:, kc, ms * P:(ms + 1) * P],
                   saWvT[:, kc, :], start=(kc == 0), stop=(kc == 3))
            nc.vector.tensor_scalar_add(
                Vsa[:, ms, :, 0:64],
                pt[:, :].rearrange("p (h d) -> p h d", h=8), 0.0)

        av_sa = [avp.tile([65, 512], F32, name=f"avs{j}", tag="av", bufs=4)
                 for j in range(4)]
        for sc in range(4):
            for hp in range(4):
                pt = ps_tile("pst")
                mm(pt[:, 0:256], KTsa[:, hp, sc * P:(sc + 1) * P],
                   QTsa[:, 2 * hp, :], start=True, stop=True,
                   skip_group_check=True)
                mm(pt[:, 256:512], KTsa[:, hp, sc * P:(sc + 1) * P],
                   QTsa[:, 2 * hp + 1, :], start=True, stop=True,
                   skip_group_check=True)
                ex = p1.tile([P, 2, 256], BF16, name="exs", tag="ex", bufs=4)
                nc.scalar.activation(
                    ex[:], pt[:, :].rearrange("p (h l) -> p h l", h=2), AF.Exp)
                for h2 in range(2):
                    h = 2 * hp + h2
                    mm(av_sa[hp][:, h2 * 256:(h2 + 1) * 256],
                       Vsa[:, sc, h, :], ex[:, h2, :],
                       start=(sc == 0), stop=(sc == 3), skip_group_check=True)

        # denominators: row 64 of each av bank -> [128L, 16] psum -> recip
        for hp in range(4):
            nc.scalar.copy(dns[64:65, hp, :], av_sa[hp][64:65, :])
        rc_ps = ps_tile("rcps")
        for hp in range(4):
            for h2 in range(2):
                for lt in range(2):
                    col = lt * 8 + 2 * hp + h2
                    mm(rc_ps[:, col:col + 1],
                       dns[64:65, hp, h2 * 256 + lt * P:h2 * 256 + (lt + 1) * P],
                       ident[64:65, 64:65], start=True, stop=True,
                       skip_group_check=True)
        recip_sa = p1.tile([P, 2, 8], F32, name="recip_sa")
        nc.vector.reciprocal(
            recip_sa[:], rc_ps[:, 0:16].rearrange("p (lt h) -> p lt h", lt=2))

        sa_acc = p1.tile([P, 2, 512], F32, name="sa_acc")
        for hp in range(4):
            for h2 in range(2):
                h = 2 * hp + h2
                U = p1.tile([64, 256], BF16, name="Usa", tag="U", bufs=3)
                nc.scalar.copy(U[:], av_sa[hp][0:64, h2 * 256:(h2 + 1) * 256])
                for lt in range(2):
                    pt = ps_tile("pproj")
                    mm(pt[:, :], U[:, lt * P:(lt + 1) * P],
                       saWoT[:, h, :], start=True, stop=True)
                    if h == 0:
                        nc.vector.tensor_scalar(sa_acc[:, lt, :], pt[:, :],
                                                recip_sa[:, lt, h:h + 1], None,
                                                op0=ALU.mult)
                    else:
                        nc.vector.scalar_tensor_tensor(
                            sa_acc[:, lt, :], pt[:, :], recip_sa[:, lt, h:h + 1],
                            sa_acc[:, lt, :], op0=ALU.mult, op1=ALU.add)

        g1 = rep(BR_LN + 6)
        be1 = rep(BR_LN + 7)
        bo_sa = rep(BR_SABO)
        for lt in range(2):
            tmp = p1.tile([P, 512], F32, name="pre3", tag="pre", bufs=3)
            nc.vector.tensor_add(tmp[:], x0[:, lt, :], sa_acc[:, lt, :])
            nc.vector.tensor_add(tmp[:], tmp[:], bo_sa[:])
            ln(x1[:, lt, :], tmp[:], g1, be1, p1)

    saw_cm.__exit__(None, None, None)

    # FFN weights: needed only at phase 3, loaded during CA (pool spans 2-3)
    ffw_cm = tc.tile_pool(name="ffw", bufs=1)
    ffw = ffw_cm.__enter__()
    W1T = wload(ffw, "W1T", [P, 4, 2048], "(kc p) m -> p kc m")
    W2T = wload(ffw, "W2T", [P, 16, 512], "(kc p) m -> p kc m")

    # ================= phase 2: cross-attention =================
    with tc.tile_pool(name="p2", bufs=1) as p2:
        # queryT = (x1 + aa)^T
        qT = p2.tile([P, 4, 256], BF16, name="qT")
        for lt in range(2):
            qpre = p2.tile([P, 512], F32, name="qpre", tag="pre", bufs=3)
            nc.vector.tensor_add(qpre[:], x1[:, lt, :], aa[:, lt, :])
            for dc in range(4):
                tp = ps_tile("tp1")
                nc.tensor.transpose(tp[:P, :P], qpre[:, dc * P:(dc + 1) * P],
                                    ident[:])
                nc.scalar.copy(qT[:, dc, lt * P:(lt + 1) * P], tp[:P, :P])
        QTca = p2.tile([P, 8, 256], BF16, name="QTca")
        nc.vector.memset(QTca[:], 0.0)
        for j in range(4):
            pt = ps_tile("pq2")
            for kc in range(4):
                mm(pt[:, :256], caWqT[:, kc, j * P:(j + 1) * P],
                   qT[:, kc, :], start=(kc == 0), stop=(kc == 3))
            nc.scalar.add(QTca[0:64, 2 * j, :], pt[0:64, :256],
                          qb_ca[0:64, j:j + 1])
            nc.scalar.add(QTca[64:128, 2 * j + 1, :], pt[64:128, :256],
                          qb_ca[64:128, j:j + 1])

        av_ca = [avp.tile([65, 512], F32, name=f"avc{j}", tag="av", bufs=4)
                 for j in range(4)]

        NSC = 16  # density chunks of 256 rows
        for sc in range(NSC):
            s0 = sc * 256
            wei = p2.tile([P, 2, 4, 2, 256], BF16, name="wei", tag="wei", bufs=2)
            nc.sync.dma_start(wei[:], din("weiT")[sc])

            first = (sc == 0)
            last = (sc == NSC - 1)
            for ms in range(2):
                for hp in range(4):
                    pt = ps_tile("pst2")
                    mm(pt[:, 0:256], ktcF[:, hp, s0 + ms * P:s0 + (ms + 1) * P],
                       QTca[:, 2 * hp, :], start=True, stop=True,
                       skip_group_check=True)
                    mm(pt[:, 256:512], ktcF[:, hp, s0 + ms * P:s0 + (ms + 1) * P],
                       QTca[:, 2 * hp + 1, :], start=True, stop=True,
                       skip_group_check=True)
                    sb = p2.tile([P, 2, 256], F32, name="sb", tag="sb", bufs=2)
                    nc.vector.tensor_add(
                        sb[:], pt[:, :].rearrange("p (h l) -> p h l", h=2),
                        wei[:, ms, hp])
                    ex = p2.tile([P, 2, 256], BF16, name="exc", tag="ex", bufs=3)
                    nc.scalar.activation(ex[:], sb[:], AF.Exp)
                    for h2 in range(2):
                        h = 2 * hp + h2
                        mm(av_ca[hp][:, h2 * 256:(h2 + 1) * 256],
                           vcF[:, 2 * sc + ms, h, :], ex[:, h2, :],
                           start=(first and ms == 0), stop=(last and ms == 1),
                           skip_group_check=True)

        for hp in range(4):
            nc.scalar.copy(dns[64:65, hp, :], av_ca[hp][64:65, :])
        rc_ps2 = ps_tile("rcps2")
        for hp in range(4):
            for h2 in range(2):
                for lt in range(2):
                    col = lt * 8 + 2 * hp + h2
                    mm(rc_ps2[:, col:col + 1],
                       dns[64:65, hp, h2 * 256 + lt * P:h2 * 256 + (lt + 1) * P],
                       ident[64:65, 64:65], start=True, stop=True,
                       skip_group_check=True)
        recip_ca = p2.tile([P, 2, 8], F32, name="recip_ca")
        nc.vector.reciprocal(
            recip_ca[:], rc_ps2[:, 0:16].rearrange("p (lt h) -> p lt h", lt=2))

        ca_acc = p2.tile([P, 2, 512], F32, name="ca_acc")
        for hp in range(4):
            for h2 in range(2):
                h = 2 * hp + h2
                U = p2.tile([64, 256], BF16, name="Uca", tag="U", bufs=3)
                nc.scalar.copy(U[:], av_ca[hp][0:64, h2 * 256:(h2 + 1) * 256])
                for lt in range(2):
                    pt = ps_tile("pproj2")
                    mm(pt[:, :], U[:, lt * P:(lt + 1) * P],
                       caWoT[:, h, :], start=True, stop=True)
                    if h == 0:
                        nc.vector.tensor_scalar(ca_acc[:, lt, :], pt[:, :],
                                                recip_ca[:, lt, h:h + 1], None,
                                                op0=ALU.mult)
                    else:
                        nc.vector.scalar_tensor_tensor(
                            ca_acc[:, lt, :], pt[:, :], recip_ca[:, lt, h:h + 1],
                            ca_acc[:, lt, :], op0=ALU.mult, op1=ALU.add)

        x2 = g.tile([P, 2, 512], F32, name="x2", tag="xres", bufs=2)
        g2 = rep(BR_LN + 8)
        be2 = rep(BR_LN + 9)
        bo_ca = rep(BR_CABO)
        for lt in range(2):
            tmp = p2.tile([P, 512], F32, name="pre4", tag="pre", bufs=3)
            nc.vector.tensor_add(tmp[:], x1[:, lt, :], ca_acc[:, lt, :])
            nc.vector.tensor_add(tmp[:], tmp[:], bo_ca[:])
            ln(x2[:, lt, :], tmp[:], g2, be2, p2)

    # ================= phase 3: FFN =================
    with tc.tile_pool(name="p3", bufs=1) as p3:
        x2T = p3.tile([P, 4, 256], BF16, name="x2T")
        for lt in range(2):
            for dc in range(4):
                tp = ps_tile("tp2")
                nc.tensor.transpose(tp[:P, :P], x2[:, lt, dc * P:(dc + 1) * P],
                                    ident[:])
                nc.scalar.copy(x2T[:, dc, lt * P:(lt + 1) * P], tp[:P, :P])

        fT = p3.tile([P, 16, 256], BF16, name="fT")
        for j in range(16):
            pt = ps_tile("pf")
            for kc in range(4):
                mm(pt[:, :256], W1T[:, kc, j * P:(j + 1) * P],
                   x2T[:, kc, :], start=(kc == 0), stop=(kc == 3))
            nc.scalar.activation(fT[:, j, :], pt[:, :256], AF.Relu,
                                 bias=b1T[:, j:j + 1])

        out_sb = g.tile([P, 2, 512], F32, name="out_sb", tag="xres", bufs=2)
        g3 = rep(BR_LN + 10)
        be3 = rep(BR_LN + 11)
        for lt in range(2):
            pt = ps_tile("pff")
            for j in range(16):
                mm(pt[:, :], fT[:, j, lt * P:(lt + 1) * P],
                   W2T[:, j, :], start=(j == 0), stop=False)
            row_bias_mm(pt, BR_B2)
            tmp = p3.tile([P, 512], F32, name="pre5", tag="pre", bufs=3)
            nc.vector.tensor_add(tmp[:], pt[:, :], x2[:, lt, :])
            ln(out_sb[:, lt, :], tmp[:], g3, be3, p3)

    nc.sync.dma_start(din("out").rearrange("(lt p) d -> p lt d", p=P), out_sb[:])

    ffw_cm.__exit__(None, None, None)
    es.close()


def _build():
    nc = bacc.Bacc("TRN2", target_bir_lowering=False, debug=False, num_devices=NC)
    specs = [
        ("msa0T", [MSA, LLOC], BF16),
        ("sgl", [LLOC, D], F32),
        ("parT", [PAIR, LLOC, NRES], F8),
        ("aa", [LLOC, D], F32),
        ("aaT", [D, NRES], BF16),
        ("aaTl", [D, LLOC], BF16),
        ("denT", [D, NDEN], BF16),
        ("kmT", [D, NDEN], BF16),
        ("weiT", [16, P, 2, 4, 2, LLOC], BF16),
        ("WmsT", [MSA, D], BF16),
        ("WpsT", [PAIR, D], BF16),
        ("saWqT", [D, D], BF16),
        ("saWkT", [D, D], BF16),
        ("saWvT", [D, D], BF16),
        ("saWoT", [D, D], BF16),
        ("caWqT", [D, D], BF16),
        ("caWkT", [D, D], BF16),
        ("caWvT", [D, D], BF16),
        ("caWoT", [D, D], BF16),
        ("W1T", [D, FF], BF16),
        ("W2T", [FF, D], BF16),
        ("qb_sa", [P, 4], F32),
        ("qb_ca", [P, 4], F32),
        ("b1T", [P, 16], F32),
        ("brows", [17, D], F32R),
        ("onesr", [1, P], F32R),
        ("ident", [P, P], F32),
    ]
    drams = {}
    for name, shape, dt in specs:
        drams[name] = nc.dram_tensor(name, shape, dt, kind="ExternalInput")
    drams["out"] = nc.dram_tensor("out", [LLOC, D], F32, kind="ExternalOutput")

    with tile.TileContext(nc) as tc:
        _emit(nc, tc, drams)
    nc.compile()
    return nc


def _prep_core_inputs(inputs, b, half):
    L0 = half * LLOC
    f32 = np.float32
    bf16 = ml_dtypes.bfloat16
    f8 = ml_dtypes.float8_e4m3

    def C(a, dt=f32):
        return np.ascontiguousarray(a, dtype=dt)

    tgt_msa = inputs["tgt_msa"]
    tgt_sgl = inputs["tgt_sgl"]
    tgt_par = inputs["tgt_par"]
    aa_embed = inputs["aa_embed"]
    density_repr = inputs["density_repr"]
    den_pos = inputs["den_pos"]
    den_wei = inputs["den_wei"]

    m = {}
    m["msa0T"] = C(tgt_msa[0, b, L0:L0 + LLOC, :].T, bf16)
    m["sgl"] = C(tgt_sgl[L0:L0 + LLOC, b])
    m["parT"] = C(tgt_par[L0:L0 + LLOC, b].transpose(2, 0, 1), f8)
    m["aa"] = C(aa_embed[L0:L0 + LLOC, b])
    m["aaT"] = C(aa_embed[:, b].T, bf16)
    m["aaTl"] = C(aa_embed[L0:L0 + LLOC, b].T, bf16)
    m["denT"] = C(density_repr[:, b].T, bf16)
    m["kmT"] = C((density_repr[:, b] + den_pos[:, b]).T, bf16)
    # weiT[sc, p, ms, hp, h2, l] = 8*den_wei[b*H + 2*hp + h2, L0+l, sc*256+ms*128+p]
    w = (8.0 * den_wei[b * H:(b + 1) * H, L0:L0 + LLOC, :]).astype(f32)
    # w: [8, 256, 4096] -> [4, 2, 256, 16, 2, 128] -> [16, 128, 2, 4, 2, 256]
    w = w.reshape(4, 2, LLOC, 16, 2, P).transpose(3, 5, 4, 0, 1, 2)
    m["weiT"] = C(w, bf16)
    return m


def _prep_shared_inputs(inputs):
    f32 = np.float32
    bf16 = ml_dtypes.bfloat16

    def C(a, dt=bf16):
        return np.ascontiguousarray(a, dtype=dt)

    m = {}
    m["WmsT"] = C(inputs["W_ms"].T)
    m["WpsT"] = C(np.asarray(inputs["W_ps"], f32).T / NRES)
    sa_W = np.asarray(inputs["sa_Wqkv"], f32)
    m["saWqT"] = C(sa_W[:D].T / 8.0)
    m["saWkT"] = C(sa_W[D:2 * D].T)
    m["saWvT"] = C(sa_W[2 * D:].T)
    m["saWoT"] = C(np.asarray(inputs["sa_Wo"], f32).T)
    ca_W = np.asarray(inputs["ca_Wqkv"], f32)
    m["caWqT"] = C(ca_W[:D].T / 8.0)
    m["caWkT"] = C(ca_W[D:2 * D].T)
    m["caWvT"] = C(ca_W[2 * D:].T)
    m["caWoT"] = C(np.asarray(inputs["ca_Wo"], f32).T)
    m["W1T"] = C(np.asarray(inputs["W1"], f32).T)
    m["W2T"] = C(np.asarray(inputs["W2"], f32).T)

    sa_b = np.asarray(inputs["sa_bqkv"], f32)
    ca_b = np.asarray(inputs["ca_bqkv"], f32)
    m["qb_sa"] = C((sa_b[:D] / 8.0).reshape(4, P).T, f32)
    m["qb_ca"] = C((ca_b[:D] / 8.0).reshape(4, P).T, f32)
    m["b1T"] = C(np.asarray(inputs["b1"], f32).reshape(16, P).T, f32)

    # V-bias folded into out-proj bias: softmax weights sum to 1
    bo_sa = np.asarray(inputs["sa_bo"], f32) + sa_b[2 * D:] @ np.asarray(
        inputs["sa_Wo"], f32).T
    bo_ca = np.asarray(inputs["ca_bo"], f32) + ca_b[2 * D:] @ np.asarray(
        inputs["ca_Wo"], f32).T

    brows = np.stack([
        inputs["b_ms"], inputs["b_ps"], bo_sa, bo_ca, inputs["b2"],
        inputs["g_ms"], inputs["be_ms"], inputs["g_ps"], inputs["be_ps"],
        inputs["g0"], inputs["be0"], inputs["g1"], inputs["be1"],
        inputs["g2"], inputs["be2"], inputs["g3"], inputs["be3"],
    ]).astype(f32)
    m["brows"] = C(brows, f32)
    m["onesr"] = np.ones((1, P), f32)
    m["ident"] = np.eye(P, dtype=f32)
    return m


def kernel(**inputs):
    global _NC, LAST_EXEC_NS
    inputs = {k: np.asarray(v) for k, v in inputs.items()}
    if _NC is None:
        _NC = _build()
    nc = _NC

    shared = _prep_shared_inputs(inputs)
    in_maps = []
    for c in range(NC):
        m = _prep_core_inputs(inputs, c // 2, c % 2)
        m.update(shared)
        in_maps.append(m)

    trace = bool(os.environ.get("BASS_TRACE"))
    res = run_bass_kernel_spmd(nc, in_maps, core_ids=list(range(NC)), trace=trace)
    LAST_EXEC_NS = res.exec_time_ns

    out = np.empty((NRES, B, D), np.float32)
    for c in range(NC):
        b, half = c // 2, c % 2
        out[half * LLOC:(half + 1) * LLOC, b] = res.results[c]["out"]
    return out


# revision 20
# speedup vs baseline: 1.9198x; 1.0297x over previous
"""Trainium2 Bass kernel for nn_CryoformerDecoderLayer.

Sharding: 8 cores = 4 batches x 2 halves of the 512 residues.
Each core computes its 256 (residue, batch) rows end-to-end; the only
cross-core exchange is a pairwise AllGather of x0 (256KB bf16) so each
pair can build full self-attention K/V for its batch.

Perf notes vs v1:
- all matmuls bf16 (FWL fast weight loads, half DMA traffic)
- parT streamed as fp8 (pair-mean branch is a ~1% perturbation)
- softmax denominators via a ones-column appended to V (row 64 of the
  attention-numerator PSUM accumulates sum(exp)) instead of 576
  moving-dim-1 matmuls
- gaussian bias added on DVE instead of an identity matmul
- K-bias dropped (softmax shift invariance, exact); V-bias folded into
  the out-proj bias on host (exact)
- all weights prefetched at kernel start
"""

import os
import numpy as np
import ml_dtypes

import concourse.bass as bass
import concourse.mybir as mybir
import concourse.bacc as bacc
import concourse.tile as tile
from concourse.bass_utils import run_bass_kernel_spmd

F32 = mybir.dt.float32
F32R = mybir.dt.float32r
BF16 = mybir.dt.bfloat16
F8 = mybir.dt.float8e4
AF = mybir.ActivationFunctionType
ALU = mybir.AluOpType

P = 128
D, H, FF, MSA, PAIR = 512, 8, 2048, 256, 128
NRES, B, NDEN = 512, 4, 4096
LLOC = 256
NC = 8
DH = D // H  # 64

# brows row indices
BR_MS, BR_PS, BR_SABO, BR_CABO, BR_B2 = range(5)
BR_LN = 5  # 5..16: g_ms, be_ms, g_ps, be_ps, g0, be0, g1, be1, g2, be2, g3, be3

LAST_EXEC_NS = None
_NC = None


def _r(ap):
    return ap.bitcast(F32R)


def _emit(nc, tc, drams):
    mm = nc.tensor.matmul

    from contextlib import ExitStack
    es = ExitStack()
    es.enter_context(nc.allow_low_precision(
        reason="bf16 matmuls are fine at rel-tol 2e-2"))
    psp = es.enter_context(tc.tile_pool(name="psp", bufs=1, space="PSUM"))
    avp = es.enter_context(tc.tile_pool(name="avp", bufs=1, space="PSUM"))
    dram = es.enter_context(tc.tile_pool(name="dram", bufs=1, space="DRAM"))
    g = es.enter_context(tc.tile_pool(name="g", bufs=1))  # global sbuf pool

    def ps_tile(name):
        return psp.tile([P, 512], F32, name=name, tag="ps", bufs=4)

    def din(name):
        return drams[name].ap()

    # ---------------- persistents / prefetched weights ----------------
    onesr = g.tile([1, P], F32R, name="onesr")
    nc.sync.dma_start(onesr[:], din("onesr")[:, :])
    ident = g.tile([P, P], F32, name="ident")
    nc.sync.dma_start(ident[:], din("ident")[:, :])

    def brow(idx):
        t = g.tile([1, 512], F32R, name=f"brow{idx}", tag="brow", bufs=4)
        nc.sync.dma_start(t[:], din("brows")[idx:idx + 1, :])
        return t

    def rep(idx):
        t = g.tile([P, 512], F32, name=f"rep{idx}", tag="rep", bufs=3)
        pt = ps_tile("prep")
        mm(pt[:, :], _r(onesr[:]), _r(brow(idx)[:]), start=True, stop=True)
        nc.scalar.copy(t[:], pt[:, :])
        return t

    def row_bias_mm(pt, idx):
        # add brows[idx] (a [512] row) onto every partition row of psum pt
        mm(pt[:, :], _r(onesr[:]), _r(brow(idx)[:]), start=False, stop=True)

    qb_sa = g.tile([P, 4], F32, name="qb_sa")
    nc.sync.dma_start(qb_sa[:], din("qb_sa")[:, :])
    qb_ca = g.tile([P, 4], F32, name="qb_ca")
    nc.sync.dma_start(qb_ca[:], din("qb_ca")[:, :])
    b1T = g.tile([P, 16], F32, name="b1T")
    nc.sync.dma_start(b1T[:], din("b1T")[:, :])

    WmsT = g.tile([P, 2, 512], BF16, name="WmsT")
    nc.sync.dma_start(WmsT[:], din("WmsT").rearrange("(kc p) d -> p kc d", p=P))

    def wload(pool, name, shape, pat, p=P):
        t = pool.tile(shape, BF16, name=name)
        nc.sync.dma_start(t[:], din(name).rearrange(pat, p=p))
        return t

    caWkT = wload(g, "caWkT", [P, 4, 512], "(kc p) m -> p kc m")
    caWvT = wload(g, "caWvT", [P, 4, 512], "(kc p) m -> p kc m")

    def ln(dst, src, g_ap, be_ap, pool):
        st6 = pool.tile([P, 6], F32, name="ln6", tag="ln6", bufs=3)
        nc.vector.bn_stats(st6[:], src)
        agg = pool.tile([P, 2], F32, name="ln2", tag="ln2", bufs=3)
        nc.vector.bn_aggr(agg[:], st6[:])
        nm = pool.tile([P, 1], F32, name="lnm", tag="lnm", bufs=3)
        nc.vector.tensor_scalar_mul(nm[:], agg[:, 0:1], -1.0)
        vr = pool.tile([P, 1], F32, name="lnv", tag="lnv", bufs=3)
        nc.vector.tensor_scalar_add(vr[:], agg[:, 1:2], 1e-5)
        rc = pool.tile([P, 1], F32, name="lnr", tag="lnr", bufs=3)
        nc.vector.reciprocal(rc[:], vr[:])
        rs = pool.tile([P, 1], F32, name="lns", tag="lns", bufs=3)
        nc.scalar.sqrt(rs[:], rc[:])
        xn = pool.tile([P, 512], F32, name="lnx", tag="lnx", bufs=3)
        nc.vector.tensor_scalar(xn[:], src, nm[:], rs[:], op0=ALU.add, op1=ALU.mult)
        nc.vector.tensor_mul(dst, xn[:], g_ap[:])
        nc.vector.tensor_add(dst, dst, be_ap[:])

    # residual-chain tiles: 2-buffer rotation (x2 reuses x0's buffer once
    # x0 is dead after the SA layernorm; out_sb reuses x1's)
    x0 = g.tile([P, 2, 512], F32, name="x0", tag="xres", bufs=2)
    x1 = g.tile([P, 2, 512], F32, name="x1", tag="xres", bufs=2)
    x0T = g.tile([P, 4, 256], BF16, name="x0T")
    dns = g.tile([65, 4, 512], F32, name="dns")

    x2 = None
    out_sb = None
    # CA K/V precomputed during phase 0 (PE is otherwise idle there)
    ktcF = g.tile([P, 4, NDEN], BF16, name="ktcF")
    vcF = g.tile([P, 32, 8, 65], BF16, name="vcF")
    nc.vector.memset(vcF[:, :, :, 64:65], 1.0)

    saw_cm = tc.tile_pool(name="saw", bufs=1)
    saw = saw_cm.__enter__()

    # ================= phase 0: pre-part =================
    with tc.tile_pool(name="p0", bufs=1) as p0:
        sgl = p0.tile([P, 2, 512], F32, name="sgl")
        nc.sync.dma_start(sgl[:], din("sgl").rearrange("(lt p) d -> p lt d", p=P))
        msa0T = p0.tile([P, 2, 256], BF16, name="msa0T")
        nc.sync.dma_start(msa0T[:], din("msa0T").rearrange("(kc p) l -> p kc l", p=P))

        xms = p0.tile([P, 2, 512], F32, name="xms")
        xps = p0.tile([P, 2, 512], F32, name="xps")

        g_ms = rep(BR_LN + 0)
        be_ms = rep(BR_LN + 1)
        for lt in range(2):
            pt = ps_tile("pms")
            for kc in range(2):
                mm(pt[:, :], msa0T[:, kc, lt * P:(lt + 1) * P],
                   WmsT[:, kc, :], start=(kc == 0), stop=False)
            row_bias_mm(pt, BR_MS)
            tmp = p0.tile([P, 512], F32, name="pre0", tag="pre", bufs=3)
            nc.vector.tensor_add(tmp[:], pt[:, :], sgl[:, lt, :])
            ln(xms[:, lt, :], tmp[:], g_ms, be_ms, p0)

        # CA K/V projections over all density chunks (fills PE during the
        # DVE/DMA-bound pair-mean stream)
        kmTap = din("kmT").rearrange("(dc p) s -> p dc s", p=P)
        denTap = din("denT").rearrange("(dc p) s -> p dc s", p=P)
        for kb in range(16):
            s0 = kb * 256
            kmc = p0.tile([P, 4, 256], BF16, name="kmc", tag="kmc", bufs=2)
            nc.scalar.dma_start(kmc[:], kmTap[:, :, s0:s0 + 256])
            dc_t = p0.tile([P, 4, 256], BF16, name="dct", tag="dct", bufs=2)
            nc.scalar.dma_start(dc_t[:], denTap[:, :, s0:s0 + 256])
            for j in range(4):
                pt = ps_tile("pk2")
                for kc in range(4):
                    mm(pt[:, :256], caWkT[:, kc, j * P:(j + 1) * P],
                       kmc[:, kc, :], start=(kc == 0), stop=(kc == 3))
                nc.scalar.copy(ktcF[:, j, s0:s0 + 256], pt[:, :256])
            for ms in range(2):
                pt = ps_tile("pv2")
                for kc in range(4):
                    mm(pt[:, :], dc_t[:, kc, ms * P:(ms + 1) * P],
                       caWvT[:, kc, :], start=(kc == 0), stop=(kc == 3))
                nc.scalar.copy(
                    vcF[:, 2 * kb + ms, :, 0:64],
                    pt[:, :].rearrange("p (h d) -> p h d", h=8))

        # pair mean: fp8 in HBM, cast to bf16 by the gpsimd DMA datapath,
        # binary-tree adds in bf16, split across DVE and GPSIMD
        pmeanT = p0.tile([P, 256], BF16, name="pmeanT")
        for i in range(32):
            pchunk = p0.tile([P, 8, 512], BF16, name="pchunk", tag="pchunk", bufs=2)
            nc.gpsimd.dma_start(pchunk[:], din("parT")[:, i * 8:(i + 1) * 8, :])
            eng = nc.gpsimd if (i % 4 == 3) else nc.vector
            t = p0.tile([P, 8, 256], BF16, name="ptree", tag="ptree", bufs=2)
            eng.tensor_add(t[:], pchunk[:, :, 0:256], pchunk[:, :, 256:512])
            w = 128
            while w >= 2:
                eng.tensor_add(t[:, :, 0:w], t[:, :, 0:w], t[:, :, w:2 * w])
                w //= 2
            eng.tensor_add(pmeanT[:, i * 8:(i + 1) * 8],
                           t[:, :, 0:1], t[:, :, 1:2])
        WpsT = g.tile([P, 512], BF16, name="WpsT")
        nc.sync.dma_start(WpsT[:], din("WpsT")[:, :])
        saWqT = wload(saw, "saWqT", [P, 4, 512], "(kc p) m -> p kc m")
        saWkT = wload(saw, "saWkT", [P, 4, 512], "(kc p) m -> p kc m")
        saWvT = wload(saw, "saWvT", [P, 4, 512], "(kc p) m -> p kc m")
        saWoT = wload(saw, "saWoT", [64, 8, 512], "(h p) m -> p h m", p=64)
        aaTl = g.tile([P, 4, 256], BF16, name="aaTl")
        nc.sync.dma_start(aaTl[:], din("aaTl").rearrange("(dc p) l -> p dc l", p=P))
        aa = g.tile([P, 2, 512], F32, name="aa")
        nc.sync.dma_start(aa[:], din("aa").rearrange("(lt p) d -> p lt d", p=P))
        caWqT = wload(g, "caWqT", [P, 4, 512], "(kc p) m -> p kc m")
        caWoT = wload(g, "caWoT", [64, 8, 512], "(h p) m -> p h m", p=64)

        g_ps = rep(BR_LN + 2)
        be_ps = rep(BR_LN + 3)
        for lt in range(2):
            pt = ps_tile("pps")
            mm(pt[:, :], pmeanT[:, lt * P:(lt + 1) * P], WpsT[:],
               start=True, stop=False)
            row_bias_mm(pt, BR_PS)
            tmp = p0.tile([P, 512], F32, name="pre1", tag="pre", bufs=3)
            nc.vector.tensor_add(tmp[:], pt[:, :], sgl[:, lt, :])
            ln(xps[:, lt, :], tmp[:], g_ps, be_ps, p0)

        g0 = rep(BR_LN + 4)
        be0 = rep(BR_LN + 5)
        for lt in range(2):
            tmp = p0.tile([P, 512], F32, name="pre2", tag="pre", bufs=3)
            nc.vector.tensor_add(tmp[:], xms[:, lt, :], xps[:, lt, :])
            ln(x0[:, lt, :], tmp[:], g0, be0, p0)

        # transpose x0 -> x0T (bf16)
        for lt in range(2):
            for dc in range(4):
                tp = ps_tile("tp0")
                nc.tensor.transpose(tp[:P, :P], x0[:, lt, dc * P:(dc + 1) * P],
                                    ident[:])
                nc.scalar.copy(x0T[:, dc, lt * P:(lt + 1) * P], tp[:P, :P])

    # ================= allgather x0T within pairs =================
    snd = dram.tile([512, 256], BF16, name="snd")
    rcv = dram.tile([2, 512, 256], BF16, name="rcv")
    nc.sync.dma_start(snd.rearrange("(dc p) l -> p dc l", p=P), x0T[:])
    nc.gpsimd.collective_compute(
        "AllGather", ALU.bypass,
        replica_groups=[[0, 1], [2, 3], [4, 5], [6, 7]],
        ins=[snd.opt()], outs=[rcv.opt()],
    )

    # ================= phase 1: self-attention =================
    with tc.tile_pool(name="p1", bufs=1) as p1:
        # local Q first: doesn't need the allgather result
        qkTl = p1.tile([P, 4, 256], BF16, name="qkTl")
        nc.vector.tensor_add(qkTl[:], x0T[:], aaTl[:])
        # Q stored per-head zero-padded to full 128 contraction rows so the
        # score matmuls never need base-partition-64 operands (bf16 matmuls
        # at base partition 64 hang the device).
        QTsa = p1.tile([P, 8, 256], BF16, name="QTsa")
        nc.vector.memset(QTsa[:], 0.0)
        for j in range(4):
            pt = ps_tile("pq")
            for kc in range(4):
                mm(pt[:, :256], saWqT[:, kc, j * P:(j + 1) * P],
                   qkTl[:, kc, :], start=(kc == 0), stop=(kc == 3))
            nc.scalar.add(QTsa[0:64, 2 * j, :], pt[0:64, :256],
                          qb_sa[0:64, j:j + 1])
            nc.scalar.add(QTsa[64:128, 2 * j + 1, :], pt[64:128, :256],
                          qb_sa[64:128, j:j + 1])

        aaT = p1.tile([P, 4, 512], BF16, name="aaT")
        nc.sync.dma_start(aaT[:], din("aaT").rearrange("(dc p) s -> p dc s", p=P))
        x0fT = p1.tile([P, 4, 512], BF16, name="x0fT")
        for r in range(2):
            nc.sync.dma_start(
                x0fT[:, :, r * 256:(r + 1) * 256],
                rcv[r, :, :].rearrange("(dc p) l -> p dc l", p=P))
        qkfT = p1.tile([P, 4, 512], BF16, name="qkfT")
        nc.vector.tensor_add(qkfT[:], x0fT[:], aaT[:])
        KTsa = p1.tile([P, 4, 512], BF16, name="KTsa")
        for j in range(4):
            pt = ps_tile("pk")
            for kc in range(4):
                mm(pt[:, :], saWkT[:, kc, j * P:(j + 1) * P],
                   qkfT[:, kc, :], start=(kc == 0), stop=(kc == 3))
            nc.vector.tensor_scalar_add(KTsa[:, j, :], pt[:, :], 0.0)
        # V with ones column appended per head (row 64 of AV = sum(exp))
        Vsa = p1.tile([P, 4, 8, 65], BF16, name="Vsa")
        nc.gpsimd.memset(Vsa[:, :, :, 64:65], 1.0)
        for ms in range(4):
            pt = ps_tile("pv")
            for kc in range(4):
                mm(pt[:, :], x0fT[:, kc, ms * P:(ms + 1) * P],
                   saWvT[:, kc, :], start=(kc == 0), stop=(kc == 3))
            nc.vector.tensor_scalar_add(
                Vsa[:, ms, :, 0:64],
                pt[:, :].rearrange("p (h d) -> p h d", h=8), 0.0)

        av_sa = [avp.tile([65, 512], F32, name=f"avs{j}", tag="av", bufs=4)
                 for j in range(4)]
        for sc in range(4):
            for hp in range(4):
                pt = ps_tile("pst")
                mm(pt[:, 0:256], KTsa[:, hp, sc * P:(sc + 1) * P],
                   QTsa[:, 2 * hp, :], start=True, stop=True,
                   skip_group_check=True)
                mm(pt[:, 256:512], KTsa[:, hp, sc * P:(sc + 1) * P],
                   QTsa[:, 2 * hp + 1, :], start=True, stop=True,
                   skip_group_check=True)
                ex = p1.tile([P, 2, 256], BF16, name="exs", tag="ex", bufs=4)
                nc.scalar.activation(
                    ex[:], pt[:, :].rearrange("p (h l) -> p h l", h=2), AF.Exp)
                for h2 in range(2):
                    h = 2 * hp + h2
                    mm(av_sa[hp][:, h2 * 256:(h2 + 1) * 256],
                       Vsa[:, sc, h, :], ex[:, h2, :],
                       start=(sc == 0), stop=(sc == 3), skip_group_check=True)

        # denominators: row 64 of each av bank -> [128L, 16] psum -> recip
        for hp in range(4):
            nc.scalar.copy(dns[64:65, hp, :], av_sa[hp][64:65, :])
        rc_ps = ps_tile("rcps")
        for hp in range(4):
            for h2 in range(2):
                for lt in range(2):
                    col = lt * 8 + 2 * hp + h2
                    mm(rc_ps[:, col:col + 1],
                       dns[64:65, hp, h2 * 256 + lt * P:h2 * 256 + (lt + 1) * P],
                       ident[64:65, 64:65], start=True, stop=True,
                       skip_group_check=True)
        recip_sa = p1.tile([P, 2, 8], F32, name="recip_sa")
        nc.vector.reciprocal(
            recip_sa[:], rc_ps[:, 0:16].rearrange("p (lt h) -> p lt h", lt=2))

        sa_acc = p1.tile([P, 2, 512], F32, name="sa_acc")
        for hp in range(4):
            for h2 in range(2):
                h = 2 * hp + h2
                U = p1.tile([64, 256], BF16, name="Usa", tag="U", bufs=3)
                nc.scalar.copy(U[:], av_sa[hp][0:64, h2 * 256:(h2 + 1) * 256])
                for lt in range(2):
                    pt = ps_tile("pproj")
                    mm(pt[:, :], U[:, lt * P:(lt + 1) * P],
                       saWoT[:, h, :], start=True, stop=True)
                    if h == 0:
                        nc.vector.tensor_scalar(sa_acc[:, lt, :], pt[:, :],
                                                recip_sa[:, lt, h:h + 1], None,
                                                op0=ALU.mult)
                    else:
                        nc.vector.scalar_tensor_tensor(
                            sa_acc[:, lt, :], pt[:, :], recip_sa[:, lt, h:h + 1],
                            sa_acc[:, lt, :], op0=ALU.mult, op1=ALU.add)

        g1 = rep(BR_LN + 6)
        be1 = rep(BR_LN + 7)
        bo_sa = rep(BR_SABO)
        for lt in range(2):
            tmp = p1.tile([P, 512], F32, name="pre3", tag="pre", bufs=3)
            nc.vector.tensor_add(tmp[:], x0[:, lt, :], sa_acc[:, lt, :])
            nc.vector.tensor_add(tmp[:], tmp[:], bo_sa[:])
            ln(x1[:, lt, :], tmp[:], g1, be1, p1)

    saw_cm.__exit__(None, None, None)

    # FFN weights: needed only at phase 3, loaded during CA (pool spans 2-3)
    ffw_cm = tc.tile_pool(name="ffw", bufs=1)
    ffw = ffw_cm.__enter__()
    W1T = wload(ffw, "W1T", [P, 4, 2048], "(kc p) m -> p kc m")
    W2T = wload(ffw, "W2T", [P, 16, 512], "(kc p) m -> p kc m")

    # ================= phase 2: cross-attention =================
    with tc.tile_pool(name="p2", bufs=1) as p2:
        # queryT = (x1 + aa)^T
        qT = p2.tile([P, 4, 256], BF16, name="qT")
        for lt in range(2):
            qpre = p2.tile([P, 512], F32, name="qpre", tag="pre", bufs=3)
            nc.vector.tensor_add(qpre[:], x1[:, lt, :], aa[:, lt, :])
            for dc in range(4):
                tp = ps_tile("tp1")
                nc.tensor.transpose(tp[:P, :P], qpre[:, dc * P:(dc + 1) * P],
                                    ident[:])
                nc.scalar.copy(qT[:, dc, lt * P:(lt + 1) * P], tp[:P, :P])
        QTca = p2.tile([P, 8, 256], BF16, name="QTca")
        nc.vector.memset(QTca[:], 0.0)
        for j in range(4):
            pt = ps_tile("pq2")
            for kc in range(4):
                mm(pt[:, :256], caWqT[:, kc, j * P:(j + 1) * P],
                   qT[:, kc, :], start=(kc == 0), stop=(kc == 3))
            nc.scalar.add(QTca[0:64, 2 * j, :], pt[0:64, :256],
                          qb_ca[0:64, j:j + 1])
            nc.scalar.add(QTca[64:128, 2 * j + 1, :], pt[64:128, :256],
                          qb_ca[64:128, j:j + 1])

        av_ca = [avp.tile([65, 512], F32, name=f"avc{j}", tag="av", bufs=4)
                 for j in range(4)]

        NSC = 16  # density chunks of 256 rows
        for sc in range(NSC):
            s0 = sc * 256
            wei = p2.tile([P, 2, 4, 2, 256], BF16, name="wei", tag="wei", bufs=2)
            nc.sync.dma_start(wei[:], din("weiT")[sc])

            first = (sc == 0)
            last = (sc == NSC - 1)
            for ms in range(2):
                for hp in range(4):
                    pt = ps_tile("pst2")
                    mm(pt[:, 0:256], ktcF[:, hp, s0 + ms * P:s0 + (ms + 1) * P],
                       QTca[:, 2 * hp, :], start=True, stop=True,
                       skip_group_check=True)
                    mm(pt[:, 256:512], ktcF[:, hp, s0 + ms * P:s0 + (ms + 1) * P],
                       QTca[:, 2 * hp + 1, :], start=True, stop=True,
                       skip_group_check=True)
                    sb = p2.tile([P, 2, 256], F32, name="sb", tag="sb", bufs=2)
                    nc.vector.tensor_add(
                        sb[:], pt[:, :].rearrange("p (h l) -> p h l", h=2),
                        wei[:, ms, hp])
                    ex = p2.tile([P, 2, 256], BF16, name="exc", tag="ex", bufs=3)
                    nc.scalar.activation(ex[:], sb[:], AF.Exp)
                    for h2 in range(2):
                        h = 2 * hp + h2
                        mm(av_ca[hp][:, h2 * 256:(h2 + 1) * 256],
                           vcF[:, 2 * sc + ms, h, :], ex[:, h2, :],
                           start=(first and ms == 0), stop=(last and ms == 1),
                           skip_group_check=True)

        for hp in range(4):
            nc.scalar.copy(dns[64:65, hp, :], av_ca[hp][64:65, :])
        rc_ps2 = ps_tile("rcps2")
        for hp in range(4):
            for h2 in range(2):
                for lt in range(2):
                    col = lt * 8 + 2 * hp + h2
                    mm(rc_ps2[:, col:col + 1],
                       dns[64:65, hp, h2 * 256 + lt * P:h2 * 256 + (lt + 1) * P],
                       ident[64:65, 64:65], start=True, stop=True,
                       skip_group_check=True)
        recip_ca = p2.tile([P, 2, 8], F32, name="recip_ca")
        nc.vector.reciprocal(
            recip_ca[:], rc_ps2[:, 0:16].rearrange("p (lt h) -> p lt h", lt=2))

        ca_acc = p2.tile([P, 2, 512], F32, name="ca_acc")
        for hp in range(4):
            for h2 in range(2):
                h = 2 * hp + h2
                U = p2.tile([64, 256], BF16, name="Uca", tag="U", bufs=3)
                nc.scalar.copy(U[:], av_ca[hp][0:64, h2 * 256:(h2 + 1) * 256])
                for lt in range(2):
                    pt = ps_tile("pproj2")
                    mm(pt[:, :], U[:, lt * P:(lt + 1) * P],
                       caWoT[:, h, :], start=True, stop=True)
                    if h == 0:
                        nc.vector.tensor_scalar(ca_acc[:, lt, :], pt[:, :],
                                                recip_ca[:, lt, h:h + 1], None,
                                                op0=ALU.mult)
                    else:
                        nc.vector.scalar_tensor_tensor(
                            ca_acc[:, lt, :], pt[:, :], recip_ca[:, lt, h:h + 1],
                            ca_acc[:, lt, :], op0=ALU.mult, op1=ALU.add)

        x2 = g.tile([P, 2, 512], F32, name="x2", tag="xres", bufs=2)
        g2 = rep(BR_LN + 8)
        be2 = rep(BR_LN + 9)
        bo_ca = rep(BR_CABO)
        for lt in range(2):
            tmp = p2.tile([P, 512], F32, name="pre4", tag="pre", bufs=3)
            nc.vector.tensor_add(tmp[:], x1[:, lt, :], ca_acc[:, lt, :])
            nc.vector.tensor_add(tmp[:], tmp[:], bo_ca[:])
            ln(x2[:, lt, :], tmp[:], g2, be2, p2)

    # ================= phase 3: FFN =================
    with tc.tile_pool(name="p3", bufs=1) as p3:
        x2T = p3.tile([P, 4, 256], BF16, name="x2T")
        for lt in range(2):
            for dc in range(4):
                tp = ps_tile("tp2")
                nc.tensor.transpose(tp[:P, :P], x2[:, lt, dc * P:(dc + 1) * P],
                                    ident[:])
                nc.scalar.copy(x2T[:, dc, lt * P:(lt + 1) * P], tp[:P, :P])

        fT = p3.tile([P, 16, 256], BF16, name="fT")
        for j in range(16):
            pt = ps_tile("pf")
            for kc in range(4):
                mm(pt[:, :256], W1T[:, kc, j * P:(j + 1) * P],
                   x2T[:, kc, :], start=(kc == 0), stop=(kc == 3))
            nc.scalar.activation(fT[:, j, :], pt[:, :256], AF.Relu,
                                 bias=b1T[:, j:j + 1])

        out_sb = g.tile([P, 2, 512], F32, name="out_sb", tag="xres", bufs=2)
        g3 = rep(BR_LN + 10)
        be3 = rep(BR_LN + 11)
        for lt in range(2):
            pt = ps_tile("pff")
            for j in range(16):
                mm(pt[:, :], fT[:, j, lt * P:(lt + 1) * P],
                   W2T[:, j, :], start=(j == 0), stop=False)
            row_bias_mm(pt, BR_B2)
            tmp = p3.tile([P, 512], F32, name="pre5", tag="pre", bufs=3)
            nc.vector.tensor_add(tmp[:], pt[:, :], x2[:, lt, :])
            ln(out_sb[:, lt, :], tmp[:], g3, be3, p3)

    nc.sync.dma_start(din("out").rearrange("(lt p) d -> p lt d", p=P), out_sb[:])

    ffw_cm.__exit__(None, None, None)
    es.close()


def _build():
    nc = bacc.Bacc("TRN2", target_bir_lowering=False, debug=False, num_devices=NC)
    specs = [
        ("msa0T", [MSA, LLOC], BF16),
        ("sgl", [LLOC, D], F32),
        ("parT", [PAIR, LLOC, NRES], F8),
        ("aa", [LLOC, D], F32),
        ("aaT", [D, NRES], BF16),
        ("aaTl", [D, LLOC], BF16),
        ("denT", [D, NDEN], BF16),
        ("kmT", [D, NDEN], BF16),
        ("weiT", [16, P, 2, 4, 2, LLOC], BF16),
        ("WmsT", [MSA, D], BF16),
        ("WpsT", [PAIR, D], BF16),
        ("saWqT", [D, D], BF16),
        ("saWkT", [D, D], BF16),
        ("saWvT", [D, D], BF16),
        ("saWoT", [D, D], BF16),
        ("caWqT", [D, D], BF16),
        ("caWkT", [D, D], BF16),
        ("caWvT", [D, D], BF16),
        ("caWoT", [D, D], BF16),
        ("W1T", [D, FF], BF16),
        ("W2T", [FF, D], BF16),
        ("qb_sa", [P, 4], F32),
        ("qb_ca", [P, 4], F32),
        ("b1T", [P, 16], F32),
        ("brows", [17, D], F32R),
        ("onesr", [1, P], F32R),
        ("ident", [P, P], F32),
    ]
    drams = {}
    for name, shape, dt in specs:
        drams[name] = nc.dram_tensor(name, shape, dt, kind="ExternalInput")
    drams["out"] = nc.dram_tensor("out", [LLOC, D], F32, kind="ExternalOutput")

    with tile.TileContext(nc) as tc:
        _emit(nc, tc, drams)
    nc.compile()
    return nc


def _prep_core_inputs(inputs, b, half):
    L0 = half * LLOC
    f32 = np.float32
    bf16 = ml_dtypes.bfloat16
    f8 = ml_dtypes.float8_e4m3

    def C(a, dt=f32):
        return np.ascontiguousarray(a, dtype=dt)

    tgt_msa = inputs["tgt_msa"]
    tgt_sgl = inputs["tgt_sgl"]
    tgt_par = inputs["tgt_par"]
    aa_embed = inputs["aa_embed"]
    density_repr = inputs["density_repr"]
    den_pos = inputs["den_pos"]
    den_wei = inputs["den_wei"]

    m = {}
    m["msa0T"] = C(tgt_msa[0, b, L0:L0 + LLOC, :].T, bf16)
    m["sgl"] = C(tgt_sgl[L0:L0 + LLOC, b])
    m["parT"] = C(tgt_par[L0:L0 + LLOC, b].transpose(2, 0, 1), f8)
    m["aa"] = C(aa_embed[L0:L0 + LLOC, b])
    m["aaT"] = C(aa_embed[:, b].T, bf16)
    m["aaTl"] = C(aa_embed[L0:L0 + LLOC, b].T, bf16)
    m["denT"] = C(density_repr[:, b].T, bf16)
    m["kmT"] = C((density_repr[:, b] + den_pos[:, b]).T, bf16)
    # weiT[sc, p, ms, hp, h2, l] = 8*den_wei[b*H + 2*hp + h2, L0+l, sc*256+ms*128+p]
    w = (8.0 * den_wei[b * H:(b + 1) * H, L0:L0 + LLOC, :]).astype(f32)
    # w: [8, 256, 4096] -> [4, 2, 256, 16, 2, 128] -> [16, 128, 2, 4, 2, 256]
    w = w.reshape(4, 2, LLOC, 16, 2, P).transpose(3, 5, 4, 0, 1, 2)
    m["weiT"] = C(w, bf16)
    return m


def _prep_shared_inputs(inputs):
    f32 = np.float32
    bf16 = ml_dtypes.bfloat16

    def C(a, dt=bf16):
        return np.ascontiguousarray(a, dtype=dt)

    m = {}
    m["WmsT"] = C(inputs["W_ms"].T)
    m["WpsT"] = C(np.asarray(inputs["W_ps"], f32).T / NRES)
    sa_W = np.asarray(inputs["sa_Wqkv"], f32)
    m["saWqT"] = C(sa_W[:D].T / 8.0)
    m["saWkT"] = C(sa_W[D:2 * D].T)
    m["saWvT"] = C(sa_W[2 * D:].T)
    m["saWoT"] = C(np.asarray(inputs["sa_Wo"], f32).T)
    ca_W = np.asarray(inputs["ca_Wqkv"], f32)
    m["caWqT"] = C(ca_W[:D].T / 8.0)
    m["caWkT"] = C(ca_W[D:2 * D].T)
    m["caWvT"] = C(ca_W[2 * D:].T)
    m["caWoT"] = C(np.asarray(inputs["ca_Wo"], f32).T)
    m["W1T"] = C(np.asarray(inputs["W1"], f32).T)
    m["W2T"] = C(np.asarray(inputs["W2"], f32).T)

    sa_b = np.asarray(inputs["sa_bqkv"], f32)
    ca_b = np.asarray(inputs["ca_bqkv"], f32)
    m["qb_sa"] = C((sa_b[:D] / 8.0).reshape(4, P).T, f32)
    m["qb_ca"] = C((ca_b[:D] / 8.0).reshape(4, P).T, f32)
    m["b1T"] = C(np.asarray(inputs["b1"], f32).reshape(16, P).T, f32)

    # V-bias folded into out-proj bias: softmax weights sum to 1
    bo_sa = np.asarray(inputs["sa_bo"], f32) + sa_b[2 * D:] @ np.asarray(
        inputs["sa_Wo"], f32).T
    bo_ca = np.asarray(inputs["ca_bo"], f32) + ca_b[2 * D:] @ np.asarray(
        inputs["ca_Wo"], f32).T

    brows = np.stack([
        inputs["b_ms"], inputs["b_ps"], bo_sa, bo_ca, inputs["b2"],
        inputs["g_ms"], inputs["be_ms"], inputs["g_ps"], inputs["be_ps"],
        inputs["g0"], inputs["be0"], inputs["g1"], inputs["be1"],
        inputs["g2"], inputs["be2"], inputs["g3"], inputs["be3"],
    ]).astype(f32)
    m["brows"] = C(brows, f32)
    m["onesr"] = np.ones((1, P), f32)
    m["ident"] = np.eye(P, dtype=f32)
    return m


def kernel(**inputs):
    global _NC, LAST_EXEC_NS
    inputs = {k: np.asarray(v) for k, v in inputs.items()}
    if _NC is None:
        _NC = _build()
    nc = _NC

    shared = _prep_shared_inputs(inputs)
    in_maps = []
    for c in range(NC):
        m = _prep_core_inputs(inputs, c // 2, c % 2)
        m.update(shared)
        in_maps.append(m)

    trace = bool(os.environ.get("BASS_TRACE"))
    res = run_bass_kernel_spmd(nc, in_maps, core_ids=list(range(NC)), trace=trace)
    LAST_EXEC_NS = res.exec_time_ns

    out = np.empty((NRES, B, D), np.float32)
    for c in range(NC):
        b, half = c // 2, c % 2
        out[half * LLOC:(half + 1) * LLOC, b] = res.results[c]["out"]
    return out


# revision 25
# speedup vs baseline: 2.0942x; 1.0909x over previous
"""Trainium2 Bass kernel for nn_CryoformerDecoderLayer.

Sharding: 8 cores = 4 batches x 2 halves of the 512 residues.
Each core computes its 256 (residue, batch) rows end-to-end; the only
cross-core exchange is a pairwise AllGather of x0 (256KB bf16) so each
pair can build full self-attention K/V for its batch.

Perf notes vs v1:
- all matmuls bf16 (FWL fast weight loads, half DMA traffic)
- parT streamed as fp8 (pair-mean branch is a ~1% perturbation)
- softmax denominators via a ones-column appended to V (row 64 of the
  attention-numerator PSUM accumulates sum(exp)) instead of 576
  moving-dim-1 matmuls
- gaussian bias added on DVE instead of an identity matmul
- K-bias dropped (softmax shift invariance, exact); V-bias folded into
  the out-proj bias on host (exact)
- all weights prefetched at kernel start
"""

import os
import numpy as np
import ml_dtypes

import concourse.bass as bass
import concourse.mybir as mybir
import concourse.bacc as bacc
import concourse.tile as tile
from concourse.bass_utils import run_bass_kernel_spmd

F32 = mybir.dt.float32
F32R = mybir.dt.float32r
BF16 = mybir.dt.bfloat16
F8 = mybir.dt.float8e4
AF = mybir.ActivationFunctionType
ALU = mybir.AluOpType

P = 128
D, H, FF, MSA, PAIR = 512, 8, 2048, 256, 128
NRES, B, NDEN = 512, 4, 4096
LLOC = 256
NC = 8
DH = D // H  # 64

# brows row indices
BR_MS, BR_PS, BR_SABO, BR_CABO, BR_B2 = range(5)
BR_LN = 5  # 5..16: g_ms, be_ms, g_ps, be_ps, g0, be0, g1, be1, g2, be2, g3, be3

LAST_EXEC_NS = None
_NC = None
STAG = int(os.environ.get('STAG', '4'))


def _r(ap):
    return ap.bitcast(F32R)


def _emit(nc, tc, drams):
    mm = nc.tensor.matmul

    from contextlib import ExitStack
    es = ExitStack()
    es.enter_context(nc.allow_low_precision(
        reason="bf16 matmuls are fine at rel-tol 2e-2"))
    psp = es.enter_context(tc.tile_pool(name="psp", bufs=1, space="PSUM"))
    avp = es.enter_context(tc.tile_pool(name="avp", bufs=1, space="PSUM"))
    dram = es.enter_context(tc.tile_pool(name="dram", bufs=1, space="DRAM"))
    g = es.enter_context(tc.tile_pool(name="g", bufs=1))  # global sbuf pool

    def ps_tile(name):
        return psp.tile([P, 512], F32, name=name, tag="ps", bufs=4)

    def din(name):
        return drams[name].ap()

    # ---------------- persistents / prefetched weights ----------------
    onesr = g.tile([1, P], F32R, name="onesr")
    nc.sync.dma_start(onesr[:], din("onesr")[:, :])
    ident = g.tile([P, P], F32, name="ident")
    nc.sync.dma_start(ident[:], din("ident")[:, :])

    def brow(idx):
        t = g.tile([1, 512], F32R, name=f"brow{idx}", tag="brow", bufs=2)
        nc.sync.dma_start(t[:], din("brows")[idx:idx + 1, :])
        return t

    def rep(idx):
        t = g.tile([P, 512], F32, name=f"rep{idx}", tag="rep", bufs=4)
        pt = ps_tile("prep")
        mm(pt[:, :], _r(onesr[:]), _r(brow(idx)[:]), start=True, stop=True)
        nc.scalar.copy(t[:], pt[:, :])
        return t

    def row_bias_mm(pt, idx):
        # add brows[idx] (a [512] row) onto every partition row of psum pt
        mm(pt[:, :], _r(onesr[:]), _r(brow(idx)[:]), start=False, stop=True)

    qb_sa = g.tile([P, 4], F32, name="qb_sa")
    nc.sync.dma_start(qb_sa[:], din("qb_sa")[:, :])
    qb_ca = g.tile([P, 4], F32, name="qb_ca")
    nc.sync.dma_start(qb_ca[:], din("qb_ca")[:, :])
    b1T = g.tile([P, 16], F32, name="b1T")
    nc.sync.dma_start(b1T[:], din("b1T")[:, :])

    WmsT = g.tile([P, 2, 512], BF16, name="WmsT")
    nc.sync.dma_start(WmsT[:], din("WmsT").rearrange("(kc p) d -> p kc d", p=P))

    def wload(pool, name, shape, pat, p=P):
        t = pool.tile(shape, BF16, name=name)
        nc.sync.dma_start(t[:], din(name).rearrange(pat, p=p))
        return t

    caWkT = wload(g, "caWkT", [P, 4, 512], "(kc p) m -> p kc m")
    caWvT = wload(g, "caWvT", [P, 4, 512], "(kc p) m -> p kc m")

    def ln(dst, src, g_ap, be_ap, pool):
        st6 = pool.tile([P, 6], F32, name="ln6", tag="ln6", bufs=3)
        nc.vector.bn_stats(st6[:], src)
        agg = pool.tile([P, 2], F32, name="ln2", tag="ln2", bufs=3)
        nc.vector.bn_aggr(agg[:], st6[:])
        nm = pool.tile([P, 1], F32, name="lnm", tag="lnm", bufs=3)
        nc.vector.tensor_scalar_mul(nm[:], agg[:, 0:1], -1.0)
        vr = pool.tile([P, 1], F32, name="lnv", tag="lnv", bufs=3)
        nc.vector.tensor_scalar_add(vr[:], agg[:, 1:2], 1e-5)
        rc = pool.tile([P, 1], F32, name="lnr", tag="lnr", bufs=3)
        nc.vector.reciprocal(rc[:], vr[:])
        rs = pool.tile([P, 1], F32, name="lns", tag="lns", bufs=3)
        nc.scalar.sqrt(rs[:], rc[:])
        xn = pool.tile([P, 512], F32, name="lnx", tag="lnx", bufs=3)
        nc.vector.tensor_scalar(xn[:], src, nm[:], rs[:], op0=ALU.add, op1=ALU.mult)
        nc.vector.tensor_mul(dst, xn[:], g_ap[:])
        nc.vector.tensor_add(dst, dst, be_ap[:])

    # residual-chain tiles: 2-buffer rotation (x2 reuses x0's buffer once
    # x0 is dead after the SA layernorm; out_sb reuses x1's)
    x0 = g.tile([P, 2, 512], F32, name="x0", tag="xres", bufs=2)
    x1 = g.tile([P, 2, 512], F32, name="x1", tag="xres", bufs=2)
    x0T = g.tile([P, 4, 256], BF16, name="x0T")
    dns = g.tile([65, 4, 512], F32, name="dns")

    x2 = None
    out_sb = None
    # CA K/V precomputed during phase 0 (PE is otherwise idle there)
    ktcF = g.tile([P, 4, NDEN], BF16, name="ktcF")
    vcF = g.tile([P, 32, 8, 65], BF16, name="vcF")
    nc.vector.memset(vcF[:, :, :, 64:65], 1.0)

    saw_cm = tc.tile_pool(name="saw", bufs=1)
    saw = saw_cm.__enter__()

    # allgather halves: strips {0,2} of the gathered L come from coll 0,
    # strips {1,3} from coll 1
    snds = [dram.tile([512, 128], BF16, name=f"snd{h}") for h in range(2)]
    rcvs = [dram.tile([2, 512, 128], BF16, name=f"rcv{h}") for h in range(2)]

    # ================= phase 0: pre-part =================
    with tc.tile_pool(name="p0", bufs=1) as p0:
        sgl = p0.tile([P, 2, 512], F32, name="sgl")
        nc.sync.dma_start(sgl[:], din("sgl").rearrange("(lt p) d -> p lt d", p=P))
        msa0T = p0.tile([P, 2, 256], BF16, name="msa0T")
        nc.sync.dma_start(msa0T[:], din("msa0T").rearrange("(kc p) l -> p kc l", p=P))

        xms = p0.tile([P, 2, 512], F32, name="xms")

        g_ms = rep(BR_LN + 0)
        be_ms = rep(BR_LN + 1)
        for lt in range(2):
            pt = ps_tile("pms")
            for kc in range(2):
                mm(pt[:, :], msa0T[:, kc, lt * P:(lt + 1) * P],
                   WmsT[:, kc, :], start=(kc == 0), stop=False)
            row_bias_mm(pt, BR_MS)
            tmp = p0.tile([P, 512], F32, name="pre0", tag="pre", bufs=3)
            nc.vector.tensor_add(tmp[:], pt[:, :], sgl[:, lt, :])
            ln(xms[:, lt, :], tmp[:], g_ms, be_ms, p0)

        # CA K/V projections over all density chunks (fills PE during the
        # DVE/DMA-bound pair-mean stream)
        kmTap = din("kmT").rearrange("(dc p) s -> p dc s", p=P)
        denTap = din("denT").rearrange("(dc p) s -> p dc s", p=P)
        for kb in range(16):
            s0 = kb * 256
            kmc = p0.tile([P, 4, 256], BF16, name="kmc", tag="kmc", bufs=2)
            nc.scalar.dma_start(kmc[:], kmTap[:, :, s0:s0 + 256])
            dc_t = p0.tile([P, 4, 256], BF16, name="dct", tag="dct", bufs=2)
            nc.scalar.dma_start(dc_t[:], denTap[:, :, s0:s0 + 256])
            for j in range(4):
                pt = ps_tile("pk2")
                for kc in range(4):
                    mm(pt[:, :256], caWkT[:, kc, j * P:(j + 1) * P],
                       kmc[:, kc, :], start=(kc == 0), stop=(kc == 3))
                nc.scalar.copy(ktcF[:, j, s0:s0 + 256], pt[:, :256])
            for ms in range(2):
                pt = ps_tile("pv2")
                for kc in range(4):
                    mm(pt[:, :], dc_t[:, kc, ms * P:(ms + 1) * P],
                       caWvT[:, kc, :], start=(kc == 0), stop=(kc == 3))
                nc.scalar.copy(
                    vcF[:, 2 * kb + ms, :, 0:64],
                    pt[:, :].rearrange("p (h d) -> p h d", h=8))

        # pair mean: fp8 in HBM, cast to bf16 by the gpsimd DMA datapath,
        # binary-tree adds in bf16 on DVE. Processed in L-halves so each
        # half's x0 can start its allgather while the other half streams.
        pmeanT = p0.tile([P, 256], BF16, name="pmeanT")
        WpsT = g.tile([P, 512], BF16, name="WpsT")
        nc.sync.dma_start(WpsT[:], din("WpsT")[:, :])
        aaTl = g.tile([P, 4, 256], BF16, name="aaTl")
        nc.sync.dma_start(aaTl[:], din("aaTl").rearrange("(dc p) l -> p dc l", p=P))
        g_ps = rep(BR_LN + 2)
        be_ps = rep(BR_LN + 3)
        g0 = rep(BR_LN + 4)
        be0 = rep(BR_LN + 5)
        for half in range(2):
            for i in range(half * 16, (half + 1) * 16):
                pchunk = p0.tile([P, 8, 512], BF16, name="pchunk", tag="pchunk", bufs=2)
                nc.gpsimd.dma_start(pchunk[:], din("parT")[:, i * 8:(i + 1) * 8, :])
                t = p0.tile([P, 8, 256], BF16, name="ptree", tag="ptree", bufs=2)
                nc.vector.tensor_add(t[:], pchunk[:, :, 0:256], pchunk[:, :, 256:512])
                w = 128
                while w >= 2:
                    nc.vector.tensor_add(t[:, :, 0:w], t[:, :, 0:w], t[:, :, w:2 * w])
                    w //= 2
                nc.vector.tensor_add(pmeanT[:, i * 8:(i + 1) * 8],
                                     t[:, :, 0:1], t[:, :, 1:2])
            pt = ps_tile("pps")
            mm(pt[:, :], pmeanT[:, half * P:(half + 1) * P], WpsT[:],
               start=True, stop=False)
            row_bias_mm(pt, BR_PS)
            tmp = p0.tile([P, 512], F32, name="pre1", tag="pre", bufs=3)
            nc.vector.tensor_add(tmp[:], pt[:, :], sgl[:, half, :])
            xpsh = p0.tile([P, 512], F32, name="xpsh", tag="xpsh", bufs=2)
            ln(xpsh[:], tmp[:], g_ps, be_ps, p0)
            tmp2 = p0.tile([P, 512], F32, name="pre2", tag="pre", bufs=3)
            nc.vector.tensor_add(tmp2[:], xms[:, half, :], xpsh[:])
            ln(x0[:, half, :], tmp2[:], g0, be0, p0)
            for dc in range(4):
                tp = ps_tile("tp0")
                nc.tensor.transpose(tp[:P, :P], x0[:, half, dc * P:(dc + 1) * P],
                                    ident[:])
                nc.scalar.copy(x0T[:, dc, half * P:(half + 1) * P], tp[:P, :P])
            nc.sync.dma_start(
                snds[half].rearrange("(dc p) l -> p dc l", p=P),
                x0T[:, :, half * P:(half + 1) * P])
            nc.gpsimd.collective_compute(
                "AllGather", ALU.bypass,
                replica_groups=[[0, 1], [2, 3], [4, 5], [6, 7]],
                ins=[snds[half].opt()], outs=[rcvs[half].opt()],
            )
            if half == 0:
                saWqT = wload(saw, "saWqT", [P, 4, 512], "(kc p) m -> p kc m")
                saWkT = wload(saw, "saWkT", [P, 4, 512], "(kc p) m -> p kc m")
                saWvT = wload(saw, "saWvT", [P, 4, 512], "(kc p) m -> p kc m")
                saWoT = wload(saw, "saWoT", [64, 8, 512], "(h p) m -> p h m", p=64)
                aa = g.tile([P, 2, 512], F32, name="aa")
                nc.sync.dma_start(aa[:], din("aa").rearrange("(lt p) d -> p lt d", p=P))
                caWqT = wload(g, "caWqT", [P, 4, 512], "(kc p) m -> p kc m")
                caWoT = wload(g, "caWoT", [64, 8, 512], "(h p) m -> p h m", p=64)


    # ================= phase 1: self-attention =================
    with tc.tile_pool(name="p1", bufs=1) as p1:
        # local Q first: doesn't need the allgather result
        qkTl = p1.tile([P, 4, 256], BF16, name="qkTl")
        nc.vector.tensor_add(qkTl[:], x0T[:], aaTl[:])
        # Q stored per-head zero-padded to full 128 contraction rows so the
        # score matmuls never need base-partition-64 operands (bf16 matmuls
        # at base partition 64 hang the device).
        QTsa = p1.tile([P, 8, 256], BF16, name="QTsa")
        nc.vector.memset(QTsa[:], 0.0)
        for j in range(4):
            pt = ps_tile("pq")
            for kc in range(4):
                mm(pt[:, :256], saWqT[:, kc, j * P:(j + 1) * P],
                   qkTl[:, kc, :], start=(kc == 0), stop=(kc == 3))
            nc.scalar.add(QTsa[0:64, 2 * j, :], pt[0:64, :256],
                          qb_sa[0:64, j:j + 1])
            nc.scalar.add(QTsa[64:128, 2 * j + 1, :], pt[64:128, :256],
                          qb_sa[64:128, j:j + 1])

        aaT = p1.tile([P, 4, 512], BF16, name="aaT")
        nc.sync.dma_start(aaT[:], din("aaT").rearrange("(dc p) s -> p dc s", p=P))
        x0fT = p1.tile([P, 4, 512], BF16, name="x0fT")
        qkfT = p1.tile([P, 4, 512], BF16, name="qkfT")
        KTsa = p1.tile([P, 4, 512], BF16, name="KTsa")
        Vsa = p1.tile([P, 4, 8, 65], BF16, name="Vsa")
        nc.gpsimd.memset(Vsa[:, :, :, 64:65], 1.0)
        av_sa = [avp.tile([65, 512], F32, name=f"avs{j}", tag="av", bufs=4)
                 for j in range(4)]

        # strips of the gathered L axis: strip s covers cols
        # [ (s//2)*256 + (s%2)*128 , +128 ) ; strips {0,2} come from coll 0.
        pend = []

        def sa_flush(n):
            while len(pend) > n:
                args = pend.pop(0)
                mm(*args[0], **args[1])

        strip_order = [0, 2, 1, 3]
        for si, s in enumerate(strip_order):
            r, hh = s // 2, s % 2
            c0 = r * 256 + hh * 128
            nc.sync.dma_start(
                x0fT[:, :, c0:c0 + 128],
                rcvs[hh][r, :, :].rearrange("(dc p) l -> p dc l", p=P))
            nc.vector.tensor_add(qkfT[:, :, c0:c0 + 128], x0fT[:, :, c0:c0 + 128],
                                 aaT[:, :, c0:c0 + 128])
            for j in range(4):
                pt = ps_tile("pk")
                for kc in range(4):
                    mm(pt[:, :128], saWkT[:, kc, j * P:(j + 1) * P],
                       qkfT[:, kc, c0:c0 + 128], start=(kc == 0), stop=(kc == 3))
                nc.vector.tensor_scalar_add(KTsa[:, j, c0:c0 + 128],
                                            pt[:, :128], 0.0)
            pt = ps_tile("pv")
            for kc in range(4):
                mm(pt[:, :], x0fT[:, kc, c0:c0 + 128],
                   saWvT[:, kc, :], start=(kc == 0), stop=(kc == 3))
            nc.scalar.copy(Vsa[:, s, :, 0:64],
                           pt[:, :].rearrange("p (h d) -> p h d", h=8))
            for hp in range(4):
                pt = ps_tile("pst")
                mm(pt[:, 0:256], KTsa[:, hp, c0:c0 + 128],
                   QTsa[:, 2 * hp, :], start=True, stop=True,
                   skip_group_check=True)
                mm(pt[:, 256:512], KTsa[:, hp, c0:c0 + 128],
                   QTsa[:, 2 * hp + 1, :], start=True, stop=True,
                   skip_group_check=True)
                ex = p1.tile([P, 2, 256], BF16, name="exs", tag="ex", bufs=4)
                nc.scalar.activation(
                    ex[:], pt[:, :].rearrange("p (h l) -> p h l", h=2), AF.Exp)
                first = (si == 0 and hp == 0)
                last = (si == 3 and hp == 3)
                for h2 in range(2):
                    h = 2 * hp + h2
                    pend.append(((av_sa[hp][:, h2 * 256:(h2 + 1) * 256],
                                  Vsa[:, s, h, :], ex[:, h2, :]),
                                 dict(start=first, stop=last,
                                      skip_group_check=True)))
                sa_flush(STAG)
        sa_flush(0)

        # denominators: row 64 of each av bank -> [128L, 16] psum -> recip
        for hp in range(4):
            nc.scalar.copy(dns[64:65, hp, :], av_sa[hp][64:65, :])
        rc_ps = ps_tile("rcps")
        for hp in range(4):
            for h2 in range(2):
                for lt in range(2):
                    col = lt * 8 + 2 * hp + h2
                    mm(rc_ps[:, col:col + 1],
                       dns[64:65, hp, h2 * 256 + lt * P:h2 * 256 + (lt + 1) * P],
                       ident[64:65, 64:65], start=True, stop=True,
                       skip_group_check=True)
        recip_sa = p1.tile([P, 2, 8], F32, name="recip_sa")
        nc.vector.reciprocal(
            recip_sa[:], rc_ps[:, 0:16].rearrange("p (lt h) -> p lt h", lt=2))

        sa_acc = p1.tile([P, 2, 512], F32, name="sa_acc")
        for hp in range(4):
            for h2 in range(2):
                h = 2 * hp + h2
                U = p1.tile([64, 256], BF16, name="Usa", tag="U", bufs=3)
                nc.scalar.copy(U[:], av_sa[hp][0:64, h2 * 256:(h2 + 1) * 256])
                for lt in range(2):
                    pt = ps_tile("pproj")
                    mm(pt[:, :], U[:, lt * P:(lt + 1) * P],
                       saWoT[:, h, :], start=True, stop=True)
                    if h == 0:
                        nc.vector.tensor_scalar(sa_acc[:, lt, :], pt[:, :],
                                                recip_sa[:, lt, h:h + 1], None,
                                                op0=ALU.mult)
                    else:
                        nc.vector.scalar_tensor_tensor(
                            sa_acc[:, lt, :], pt[:, :], recip_sa[:, lt, h:h + 1],
                            sa_acc[:, lt, :], op0=ALU.mult, op1=ALU.add)

        g1 = rep(BR_LN + 6)
        be1 = rep(BR_LN + 7)
        bo_sa = rep(BR_SABO)
        for lt in range(2):
            tmp = p1.tile([P, 512], F32, name="pre3", tag="pre", bufs=3)
            nc.vector.tensor_add(tmp[:], x0[:, lt, :], sa_acc[:, lt, :])
            nc.vector.tensor_add(tmp[:], tmp[:], bo_sa[:])
            ln(x1[:, lt, :], tmp[:], g1, be1, p1)

    saw_cm.__exit__(None, None, None)

    # FFN weights: needed only at phase 3, loaded during CA (pool spans 2-3)
    ffw_cm = tc.tile_pool(name="ffw", bufs=1)
    ffw = ffw_cm.__enter__()
    W1T = wload(ffw, "W1T", [P, 4, 2048], "(kc p) m -> p kc m")
    W2T = wload(ffw, "W2T", [P, 16, 512], "(kc p) m -> p kc m")

    # ================= phase 2: cross-attention =================
    with tc.tile_pool(name="p2", bufs=1) as p2:
        # queryT = (x1 + aa)^T
        qT = p2.tile([P, 4, 256], BF16, name="qT")
        for lt in range(2):
            qpre = p2.tile([P, 512], F32, name="qpre", tag="pre", bufs=3)
            nc.vector.tensor_add(qpre[:], x1[:, lt, :], aa[:, lt, :])
            for dc in range(4):
                tp = ps_tile("tp1")
                nc.tensor.transpose(tp[:P, :P], qpre[:, dc * P:(dc + 1) * P],
                                    ident[:])
                nc.scalar.copy(qT[:, dc, lt * P:(lt + 1) * P], tp[:P, :P])
        QTca = p2.tile([P, 8, 256], BF16, name="QTca")
        nc.vector.memset(QTca[:], 0.0)
        for j in range(4):
            pt = ps_tile("pq2")
            for kc in range(4):
                mm(pt[:, :256], caWqT[:, kc, j * P:(j + 1) * P],
                   qT[:, kc, :], start=(kc == 0), stop=(kc == 3))
            nc.scalar.add(QTca[0:64, 2 * j, :], pt[0:64, :256],
                          qb_ca[0:64, j:j + 1])
            nc.scalar.add(QTca[64:128, 2 * j + 1, :], pt[64:128, :256],
                          qb_ca[64:128, j:j + 1])

        av_ca = [avp.tile([65, 512], F32, name=f"avc{j}", tag="av", bufs=4)
                 for j in range(4)]

        NSC = 16  # density chunks of 256 rows
        pend2 = []

        def ca_flush(n):
            while len(pend2) > n:
                args = pend2.pop(0)
                mm(*args[0], **args[1])

        for sc in range(NSC):
            s0 = sc * 256
            wei = p2.tile([P, 2, 4, 2, 256], BF16, name="wei", tag="wei", bufs=2)
            nc.sync.dma_start(wei[:], din("weiT")[sc])

            first = (sc == 0)
            last = (sc == NSC - 1)
            for ms in range(2):
                for hp in range(4):
                    pt = ps_tile("pst2")
                    mm(pt[:, 0:256], ktcF[:, hp, s0 + ms * P:s0 + (ms + 1) * P],
                       QTca[:, 2 * hp, :], start=True, stop=True,
                       skip_group_check=True)
                    mm(pt[:, 256:512], ktcF[:, hp, s0 + ms * P:s0 + (ms + 1) * P],
                       QTca[:, 2 * hp + 1, :], start=True, stop=True,
                       skip_group_check=True)
                    sb = p2.tile([P, 2, 256], BF16, name="sb", tag="sb", bufs=3)
                    nc.vector.tensor_add(
                        sb[:], pt[:, :].rearrange("p (h l) -> p h l", h=2),
                        wei[:, ms, hp])
                    ex = p2.tile([P, 2, 256], BF16, name="exc", tag="ex", bufs=3)
                    nc.scalar.activation(ex[:], sb[:], AF.Exp)
                    for h2 in range(2):
                        h = 2 * hp + h2
                        pend2.append(((av_ca[hp][:, h2 * 256:(h2 + 1) * 256],
                                       vcF[:, 2 * sc + ms, h, :], ex[:, h2, :]),
                                      dict(start=(first and ms == 0),
                                           stop=(last and ms == 1),
                                           skip_group_check=True)))
                    ca_flush(STAG)
        ca_flush(0)

        for hp in range(4):
            nc.scalar.copy(dns[64:65, hp, :], av_ca[hp][64:65, :])
        rc_ps2 = ps_tile("rcps2")
        for hp in range(4):
            for h2 in range(2):
                for lt in range(2):
                    col = lt * 8 + 2 * hp + h2
                    mm(rc_ps2[:, col:col + 1],
                       dns[64:65, hp, h2 * 256 + lt * P:h2 * 256 + (lt + 1) * P],
                       ident[64:65, 64:65], start=True, stop=True,
                       skip_group_check=True)
        recip_ca = p2.tile([P, 2, 8], F32, name="recip_ca")
        nc.vector.reciprocal(
            recip_ca[:], rc_ps2[:, 0:16].rearrange("p (lt h) -> p lt h", lt=2))

        ca_acc = p2.tile([P, 2, 512], F32, name="ca_acc")
        for hp in range(4):
            for h2 in range(2):
                h = 2 * hp + h2
                U = p2.tile([64, 256], BF16, name="Uca", tag="U", bufs=3)
                nc.scalar.copy(U[:], av_ca[hp][0:64, h2 * 256:(h2 + 1) * 256])
                for lt in range(2):
                    pt = ps_tile("pproj2")
                    mm(pt[:, :], U[:, lt * P:(lt + 1) * P],
                       caWoT[:, h, :], start=True, stop=True)
                    if h == 0:
                        nc.vector.tensor_scalar(ca_acc[:, lt, :], pt[:, :],
                                                recip_ca[:, lt, h:h + 1], None,
                                                op0=ALU.mult)
                    else:
                        nc.vector.scalar_tensor_tensor(
                            ca_acc[:, lt, :], pt[:, :], recip_ca[:, lt, h:h + 1],
                            ca_acc[:, lt, :], op0=ALU.mult, op1=ALU.add)

        x2 = g.tile([P, 2, 512], F32, name="x2", tag="xres", bufs=2)
        g2 = rep(BR_LN + 8)
        be2 = rep(BR_LN + 9)
        bo_ca = rep(BR_CABO)
        for lt in range(2):
            tmp = p2.tile([P, 512], F32, name="pre4", tag="pre", bufs=3)
            nc.vector.tensor_add(tmp[:], x1[:, lt, :], ca_acc[:, lt, :])
            nc.vector.tensor_add(tmp[:], tmp[:], bo_ca[:])
            ln(x2[:, lt, :], tmp[:], g2, be2, p2)

    # ================= phase 3: FFN =================
    with tc.tile_pool(name="p3", bufs=1) as p3:
        x2T = p3.tile([P, 4, 256], BF16, name="x2T")
        for lt in range(2):
            for dc in range(4):
                tp = ps_tile("tp2")
                nc.tensor.transpose(tp[:P, :P], x2[:, lt, dc * P:(dc + 1) * P],
                                    ident[:])
                nc.scalar.copy(x2T[:, dc, lt * P:(lt + 1) * P], tp[:P, :P])

        fT = p3.tile([P, 16, 256], BF16, name="fT")
        for j in range(16):
            pt = ps_tile("pf")
            for kc in range(4):
                mm(pt[:, :256], W1T[:, kc, j * P:(j + 1) * P],
                   x2T[:, kc, :], start=(kc == 0), stop=(kc == 3))
            nc.scalar.activation(fT[:, j, :], pt[:, :256], AF.Relu,
                                 bias=b1T[:, j:j + 1])

        out_sb = g.tile([P, 2, 512], F32, name="out_sb", tag="xres", bufs=2)
        g3 = rep(BR_LN + 10)
        be3 = rep(BR_LN + 11)
        for lt in range(2):
            pt = ps_tile("pff")
            for j in range(16):
                mm(pt[:, :], fT[:, j, lt * P:(lt + 1) * P],
                   W2T[:, j, :], start=(j == 0), stop=False)
            row_bias_mm(pt, BR_B2)
            tmp = p3.tile([P, 512], F32, name="pre5", tag="pre", bufs=3)
            nc.vector.tensor_add(tmp[:], pt[:, :], x2[:, lt, :])
            ln(out_sb[:, lt, :], tmp[:], g3, be3, p3)

    nc.sync.dma_start(din("out").rearrange("(lt p) d -> p lt d", p=P), out_sb[:])

    ffw_cm.__exit__(None, None, None)
    es.close()


def _build():
    nc = bacc.Bacc("TRN2", target_bir_lowering=False, debug=False, num_devices=NC)
    specs = [
        ("msa0T", [MSA, LLOC], BF16),
        ("sgl", [LLOC, D], F32),
        ("parT", [PAIR, LLOC, NRES], F8),
        ("aa", [LLOC, D], F32),
        ("aaT", [D, NRES], BF16),
        ("aaTl", [D, LLOC], BF16),
        ("denT", [D, NDEN], BF16),
        ("kmT", [D, NDEN], BF16),
        ("weiT", [16, P, 2, 4, 2, LLOC], BF16),
        ("WmsT", [MSA, D], BF16),
        ("WpsT", [PAIR, D], BF16),
        ("saWqT", [D, D], BF16),
        ("saWkT", [D, D], BF16),
        ("saWvT", [D, D], BF16),
        ("saWoT", [D, D], BF16),
        ("caWqT", [D, D], BF16),
        ("caWkT", [D, D], BF16),
        ("caWvT", [D, D], BF16),
        ("caWoT", [D, D], BF16),
        ("W1T", [D, FF], BF16),
        ("W2T", [FF, D], BF16),
        ("qb_sa", [P, 4], F32),
        ("qb_ca", [P, 4], F32),
        ("b1T", [P, 16], F32),
        ("brows", [17, D], F32R),
        ("onesr", [1, P], F32R),
        ("ident", [P, P], F32),
    ]
    drams = {}
    for name, shape, dt in specs:
        drams[name] = nc.dram_tensor(name, shape, dt, kind="ExternalInput")
    drams["out"] = nc.dram_tensor("out", [LLOC, D], F32, kind="ExternalOutput")

    with tile.TileContext(nc) as tc:
        _emit(nc, tc, drams)
    nc.compile()
    return nc


def _prep_core_inputs(inputs, b, half):
    L0 = half * LLOC
    f32 = np.float32
    bf16 = ml_dtypes.bfloat16
    f8 = ml_dtypes.float8_e4m3

    def C(a, dt=f32):
        return np.ascontiguousarray(a, dtype=dt)

    tgt_msa = inputs["tgt_msa"]
    tgt_sgl = inputs["tgt_sgl"]
    tgt_par = inputs["tgt_par"]
    aa_embed = inputs["aa_embed"]
    density_repr = inputs["density_repr"]
    den_pos = inputs["den_pos"]
    den_wei = inputs["den_wei"]

    m = {}
    m["msa0T"] = C(tgt_msa[0, b, L0:L0 + LLOC, :].T, bf16)
    m["sgl"] = C(tgt_sgl[L0:L0 + LLOC, b])
    m["parT"] = C(tgt_par[L0:L0 + LLOC, b].transpose(2, 0, 1), f8)
    m["aa"] = C(aa_embed[L0:L0 + LLOC, b])
    m["aaT"] = C(aa_embed[:, b].T, bf16)
    m["aaTl"] = C(aa_embed[L0:L0 + LLOC, b].T, bf16)
    m["denT"] = C(density_repr[:, b].T, bf16)
    m["kmT"] = C((density_repr[:, b] + den_pos[:, b]).T, bf16)
    # weiT[sc, p, ms, hp, h2, l] = 8*den_wei[b*H + 2*hp + h2, L0+l, sc*256+ms*128+p]
    w = (8.0 * den_wei[b * H:(b + 1) * H, L0:L0 + LLOC, :]).astype(f32)
    # w: [8, 256, 4096] -> [4, 2, 256, 16, 2, 128] -> [16, 128, 2, 4, 2, 256]
    w = w.reshape(4, 2, LLOC, 16, 2, P).transpose(3, 5, 4, 0, 1, 2)
    m["weiT"] = C(w, bf16)
    return m


def _prep_shared_inputs(inputs):
    f32 = np.float32
    bf16 = ml_dtypes.bfloat16

    def C(a, dt=bf16):
        return np.ascontiguousarray(a, dtype=dt)

    m = {}
    m["WmsT"] = C(inputs["W_ms"].T)
    m["WpsT"] = C(np.asarray(inputs["W_ps"], f32).T / NRES)
    sa_W = np.asarray(inputs["sa_Wqkv"], f32)
    m["saWqT"] = C(sa_W[:D].T / 8.0)
    m["saWkT"] = C(sa_W[D:2 * D].T)
    m["saWvT"] = C(sa_W[2 * D:].T)
    m["saWoT"] = C(np.asarray(inputs["sa_Wo"], f32).T)
    ca_W = np.asarray(inputs["ca_Wqkv"], f32)
    m["caWqT"] = C(ca_W[:D].T / 8.0)
    m["caWkT"] = C(ca_W[D:2 * D].T)
    m["caWvT"] = C(ca_W[2 * D:].T)
    m["caWoT"] = C(np.asarray(inputs["ca_Wo"], f32).T)
    m["W1T"] = C(np.asarray(inputs["W1"], f32).T)
    m["W2T"] = C(np.asarray(inputs["W2"], f32).T)

    sa_b = np.asarray(inputs["sa_bqkv"], f32)
    ca_b = np.asarray(inputs["ca_bqkv"], f32)
    m["qb_sa"] = C((sa_b[:D] / 8.0).reshape(4, P).T, f32)
    m["qb_ca"] = C((ca_b[:D] / 8.0).reshape(4, P).T, f32)
    m["b1T"] = C(np.asarray(inputs["b1"], f32).reshape(16, P).T, f32)

    # V-bias folded into out-proj bias: softmax weights sum to 1
    bo_sa = np.asarray(inputs["sa_bo"], f32) + sa_b[2 * D:] @ np.asarray(
        inputs["sa_Wo"], f32).T
    bo_ca = np.asarray(inputs["ca_bo"], f32) + ca_b[2 * D:] @ np.asarray(
        inputs["ca_Wo"], f32).T

    brows = np.stack([
        inputs["b_ms"], inputs["b_ps"], bo_sa, bo_ca, inputs["b2"],
        inputs["g_ms"], inputs["be_ms"], inputs["g_ps"], inputs["be_ps"],
        inputs["g0"], inputs["be0"], inputs["g1"], inputs["be1"],
        inputs["g2"], inputs["be2"], inputs["g3"], inputs["be3"],
    ]).astype(f32)
    m["brows"] = C(brows, f32)
    m["onesr"] = np.ones((1, P), f32)
    m["ident"] = np.eye(P, dtype=f32)
    return m


def kernel(**inputs):
    global _NC, LAST_EXEC_NS
    inputs = {k: np.asarray(v) for k, v in inputs.items()}
    if _NC is None:
        _NC = _build()
    nc = _NC

    shared = _prep_shared_inputs(inputs)
    in_maps = []
    for c in range(NC):
        m = _prep_core_inputs(inputs, c // 2, c % 2)
        m.update(shared)
        in_maps.append(m)

    trace = bool(os.environ.get("BASS_TRACE"))
    res = run_bass_kernel_spmd(nc, in_maps, core_ids=list(range(NC)), trace=trace)
    LAST_EXEC_NS = res.exec_time_ns

    out = np.empty((NRES, B, D), np.float32)
    for c in range(NC):
        b, half = c // 2, c % 2
        out[half * LLOC:(half + 1) * LLOC, b] = res.results[c]["out"]
    return out
